# revision 70
# baseline (speedup 1.0000x reference)
"""Causal self-attention (B=4 T=2048 C=768 H=12) on 8 trn2 cores — v3.

Sharding: core = (batch b, head-group g), g in {0,1} covering 6 heads
(3 pairs).  Host sums the two partial c_proj outputs per batch and adds
the exact bias row (b_proj + bv @ w_proj; softmax rows sum to 1).

v3 design (vs v2, 140850 -> 130486 ns):
  * hi/lo fp8 QKV projections: host splits x and 32*w_attn into e4m3
    hi + residual-lo (subnormal) pairs; three 256-contraction DoubleRow
    chains (hi.hi + hi.lo + lo.hi) in one psum accumulation run at 2x
    the fp16 rate with ~0.14% error.  1/32 unapplied in the evacuations.
  * S^T = K^T.Q per (pair, head) via fp8e4m3 DoubleRow matmuls at 0.5
    cycles/row (contraction-64 as 2 broadcast slots; exp scale 0.0625
    absorbs the doubling).
  * S evacuation split across engines (pool is barred from PSUM): act
    does true exp for ~58% of elements, dve does a 1-op fp16-bitcast
    schraudolph exp (tensor_scalar mult/add into an int16-bitcast view,
    1.8% RMS on its share; tile-scattered so end-to-end cost is ~5e-4).
    Per-q act shares lean higher for late tiles (dve carries retire).
  * causal masks on the pool engine (SBUF-only); y normalization via
    reciprocal + per-subtile tensor_scalar on dve; y^T via XBAR
    transpose-DMAs on the sync queue (PE/DVE freed); c_proj evacuation
    on dve early / act late (act idles in the tail).
  * chunk-queue software pipelining with a small (160ns) per-S-batch
    drain budget; weight/x DMA issue order tuned against the single
    serializing DMA_ENGINES device (K-hi, x0, Q-hi, los, V, wp).
"""

from collections import deque
from contextlib import ExitStack

import numpy as np

import concourse.bass as bass
import concourse.mybir as mybir
import concourse.tile as tile
from concourse import bacc
from concourse.masks import make_upper_triangular, make_identity

AF = mybir.ActivationFunctionType
ALU = mybir.AluOpType
F32 = mybir.dt.float32
F16 = mybir.dt.float16
F8 = mybir.dt.float8e4
I16 = mybir.dt.int16
DR = mybir.MatmulPerfMode.DoubleRow

# fp16-bitcast schraudolph exp approximation: exp(s*scale) ~=
# bitcast16(int16(A_SCHR*s + B_SCHR)).  C=-0.0575 minimizes RMS rel err
# (1.78%); +0.5 turns the trunc-on-int-store into round-to-nearest.
SCHR_SCALE = 0.0625          # fp8-DR doubled psum units
A_SCHR = float(np.float32(1024.0 * 1.4426950408889634 * SCHR_SCALE))
B_SCHR = float(np.float32(15 * 1024 + 1024.0 * (-0.0575) + 0.5))

# engine shares for the S-stream evacuation (fraction of elements).
# pool cannot touch PSUM (hard BIR rule), so the split is act (true exp,
# 0.833/elem) vs dve (schraudolph, 1.04/elem on fp32 psum).
SHARE = {"act": 0.60, "dve": 0.40}
# per-q act-share overrides: late (big) stages run while dve also carries
# the retire stream, so they lean harder on act
SHARE_BY_Q = {0: 0.55, 1: 0.60, 2: 0.60, 3: 0.62}
ALTERNATE = False   # strict act/dve alternation for S evacuation

C = 768          # model dim
D = 64           # head dim
HG = 6           # heads per core
NP = 3           # head pairs per core
GC = HG * D      # 384 group channels
CT = C // 128    # 6 contraction tiles
QBLK = 512       # query tile (psum bank)
KBLK = 128       # key tile

S_FP8 = True     # fp8e4m3 DoubleRow for the S matmul (else fp16)
WARMUP = 12      # PE p-state warmup matmuls at startup
HOLDOUT = None   # stage held to the end to shorten the tail (None = off)
DRAIN_NS = 160.0  # default per-S-batch PE-work drain budget (gates override)


def build_nc(T=2048, s_fp8=S_FP8, gates=None):
    NQ = T // QBLK
    NK = T // KBLK
    nc = bacc.Bacc(None)

    # x and w_attn arrive as hi/lo fp8 pairs: x ~= xth + xtl (lo holds the
    # quantization residual, subnormal-heavy), w_attn scaled by 32 so its
    # hi part uses the e4m3 normal range; the 1/32 is unapplied in the
    # q/k/v psum evacuations.  Projections run as three fp8 DoubleRow
    # chains (hi.hi + hi.lo + lo.hi) at 2x the fp16 matmul rate.
    xth_d = nc.dram_tensor("xth", [C, T], F8, kind="ExternalInput")
    xtl_d = nc.dram_tensor("xtl", [C, T], F8, kind="ExternalInput")
    # per-projection weights, host-PRE-SHUFFLED to [r=128, (c2 j m)] so
    # both DMA sides are 2304B-contiguous (full-rate; a 384B-row slice
    # transfer pays the <512B half-rate penalty)
    wqkv_d = {}
    for nm in ("wqh", "wql", "wkh", "wkl", "wvh", "wvl"):
        wqkv_d[nm] = nc.dram_tensor(nm, [128, 6 * GC], F8,
                                    kind="ExternalInput")
    wp_d = nc.dram_tensor("wp", [GC, C], F16, kind="ExternalInput")
    out_d = nc.dram_tensor("out", [T, C], F16, kind="ExternalOutput")
    W_UNSCALE = 1.0 / 32.0

    qk_dt = F8 if s_fp8 else F16
    exp_scale = 0.0625 if s_fp8 else 0.125

    with ExitStack() as ctx:
        tc = ctx.enter_context(tile.TileContext(nc))
        const = ctx.enter_context(tc.tile_pool(name="const", bufs=1))
        big = ctx.enter_context(tc.tile_pool(name="big", bufs=1))
        xtp = ctx.enter_context(tc.tile_pool(name="xtp", bufs=4))
        ptp = ctx.enter_context(tc.tile_pool(name="ptp", bufs=3))
        yqp = ctx.enter_context(tc.tile_pool(name="yqp", bufs=3))
        recp = ctx.enter_context(tc.tile_pool(name="recp", bufs=3))
        ytp = ctx.enter_context(tc.tile_pool(name="ytp", bufs=4))
        obp = ctx.enter_context(tc.tile_pool(name="obp", bufs=2))
        psS = ctx.enter_context(tc.tile_pool(name="psS", bufs=2, space="PSUM"))
        psY = ctx.enter_context(tc.tile_pool(name="psY", bufs=2, space="PSUM"))
        psQ = ctx.enter_context(tc.tile_pool(name="psQ", bufs=2, space="PSUM"))

        # constants.  ident first: the PE warmup matmuls depend on it, and
        # everything later on the pool queue (SWDGE descriptor generation,
        # mask builds) would delay it by ~3us.
        ident = const.tile([128, 128], F16)
        make_identity(nc, ident)
        mask2 = const.tile([128, 2, KBLK], F16)   # causal keep-mask, 2 heads
        make_upper_triangular(nc, mask2[:, 0, :], val=1.0, diag=True)
        make_upper_triangular(nc, mask2[:, 1, :], val=1.0, diag=True)

        # persistent.  weight/x layout for 256-contraction DoubleRow:
        # [r=128, ct2 in 3, j in 2, cols]; contraction c = ct2*256+j*128+r.
        CT2 = 3
        w8 = {}   # (proj, hl) -> [128, CT2, 2, GC] tile
        for proj in "qkv":
            for hl in (0, 1):
                w8[proj, hl] = big.tile([128, CT2, 2, GC], F8,
                                        name=f"w8{proj}{hl}")
        wp = big.tile([128, NP, C], F16)
        kt8 = big.tile([128, NP, T], qk_dt)
        qt8 = big.tile([128, NQ, NP, QBLK], qk_dt)
        # V with trailing ones column per (ktile, head): [k, 66] rows
        vs = big.tile([128, NK, HG, D + 2], F16)
        nc.gpsimd.memset(vs[:, :, :, D:D + 1], 1.0)

        xt_r = {0: xth_d[:, :].rearrange("(c2 j r) t -> r c2 j t",
                                         r=128, j=2),
                1: xtl_d[:, :].rearrange("(c2 j r) t -> r c2 j t",
                                         r=128, j=2)}
        # (xw)-chain list: (x hi/lo, w hi/lo)
        CHAINS = ((0, 0), (0, 1), (1, 0))
        wp_r = wp_d[:, :].rearrange("(p r) e -> r p e", r=128)
        out_r = out_d[:, :].rearrange("(q tt r) e -> q r tt e", tt=QBLK // KBLK,
                                      r=128)

        def dr_ap(ap):
            """[64, N] fp8 AP -> [64, 2, N] stride-0 DoubleRow operand."""
            return ap.unsqueeze(1).broadcast_to(
                [ap.shape[0], 2] + list(ap.shape[1:]))

        # ---- chunk helpers (each chunk = (callable, pe_ns), issued later) --
        PE_NS = 1.0 / 2.4   # ns per PE cycle at full speed
        work = deque()

        pe_ord = [0]   # PE event ordinal (Ldweights+Matmult pairs)

        def MM(*a, **k):
            pe_ord[0] += 2
            return nc.tensor.matmul(*a, **k)

        xtqs = {}

        def qk_chunks(q):
            """DMA + Q/K projection groups for tile q (critical early path)."""
            qs = q * QBLK

            def dma_x():
                xtq = xtp.tile([128, 2, CT2, 2, QBLK], F8, tag="xtq",
                               name="xtq")
                xtqs[q] = xtq
                for hl in (0, 1):   # hi first: half-0 chains need only hi
                    nc.sync.dma_start(out=xtq[:, hl, :, :, :],
                                      in_=xt_r[hl][:, :, :, qs:qs + QBLK])

            chunks = [(dma_x, 0.0, f"dx:{q}")]

            pss = {}

            def qk_part(p, which, half):
                if half == 0:
                    pss[(p, which)] = psQ.tile([128, QBLK], F32, tag="pq",
                                               name="pqk")
                ps = pss[(p, which)]
                proj = "q" if which == 0 else "k"
                # half 0: the hi.hi chain; half 1: the two cross chains
                chains = CHAINS[0:1] if half == 0 else CHAINS[1:3]
                for ci, (xs, ws) in enumerate(chains):
                    for ct2 in range(CT2):
                        first = half == 0 and ci == 0 and ct2 == 0
                        last = half == 1 and ci == len(chains) - 1 \
                            and ct2 == CT2 - 1
                        MM(ps, lhsT=w8[proj, ws][:, ct2, :,
                                                 p * 128:(p + 1) * 128],
                           rhs=xtqs[q][:, xs, ct2, :, :],
                           start=first, stop=last, perf_mode=DR)
                if half == 1:
                    del pss[(p, which)]
                    if which == 0:
                        nc.scalar.mul(qt8[:, q, p, :], ps, W_UNSCALE)
                    else:
                        nc.scalar.mul(kt8[:, p, qs:qs + QBLK], ps, W_UNSCALE)

            # half-0 (hi.hi) chains first within each pair: they need only
            # the hi tensors, which DMA-land first
            for p in range(NP):
                for half in (0, 1):
                    for which in (1, 0):   # K first
                        chunks.append(
                            (lambda p=p, w=which, h=half: qk_part(p, w, h),
                             (3 if half == 0 else 6) * 256 * PE_NS,
                             f"qk:{q}:{p}:{which}" if half else
                             f"qka:{q}:{p}:{which}"))
            return chunks

        def v_chunks(q):
            """V projection groups for tile q (needed by PV, not by S/exp)."""

            pss = {}

            def v_part(kl, half):
                k_i = 4 * q + kl
                if half == 0:
                    pss[kl] = psQ.tile([128, QBLK], F32, tag="pq", name="pv")
                ps = pss[kl]
                chains = CHAINS[0:1] if half == 0 else CHAINS[1:3]
                for ci, (xs, ws) in enumerate(chains):
                    for ct2 in range(CT2):
                        first = half == 0 and ci == 0 and ct2 == 0
                        last = half == 1 and ci == len(chains) - 1 \
                            and ct2 == CT2 - 1
                        MM(ps[:, 0:GC],
                           lhsT=xtqs[q][:, xs, ct2, :,
                                        kl * KBLK:(kl + 1) * KBLK],
                           rhs=w8["v", ws][:, ct2, :, :],
                           start=first, stop=last, perf_mode=DR)
                if half == 1:
                    del pss[kl]
                    nc.scalar.mul(
                        vs[:, k_i, :, 0:D],
                        ps[:, 0:GC].rearrange("r (h d) -> r h d", d=D),
                        W_UNSCALE)

            return [(lambda kl=kl, h=h: v_part(kl, h),
                     (3 if h == 0 else 6) * (GC // 2) * PE_NS,
                     f"v:{q}:{kl}" if h else f"va:{q}:{kl}")
                    for kl in range(4) for h in (0, 1)]

        # engine assignment for S-stream evacuation.  strict alternation:
        # consecutive evacuations on the same engine serialize (~1.1us
        # each) while alternating ones overlap, halving the S-stream
        # period.  ALT_PATTERN cycles act/dve; deficit mode weights by
        # SHARE instead.
        eng_credit = {"act": 0.0, "dve": 0.0}
        alt_state = [0]

        def pick_engine(elems, q=None):
            if ALTERNATE:
                e = ("act", "dve")[alt_state[0] % 2]
                alt_state[0] += 1
                return e
            a = SHARE_BY_Q.get(q, SHARE["act"])
            shares = {"act": a, "dve": 1.0 - a}
            for e in eng_credit:
                eng_credit[e] += shares[e] * elems
            best = max(eng_credit, key=lambda e: eng_credit[e])
            eng_credit[best] -= elems
            return best

        def s_evac(eng, dst, src):
            if eng == "act":
                nc.scalar.activation(dst, src, AF.Exp, scale=exp_scale)
            else:
                nc.vector.tensor_scalar(dst.bitcast(I16), src,
                                        A_SCHR, B_SCHR,
                                        op0=ALU.mult, op1=ALU.add)

        def s_batch(q, p, k_i, pt):
            """S^T (both heads) + exp + mask for one k-tile."""
            col0 = max(k_i - 4 * q, 0) * KBLK
            st = psS.tile([128, 2, QBLK], F32, tag="st", name="st")
            for s in range(2):
                hoff = D * s
                lhsT = kt8[hoff:hoff + D, p, k_i * KBLK:(k_i + 1) * KBLK]
                rhs = qt8[hoff:hoff + D, q, p, col0:QBLK]
                MM(st[:, s, col0:QBLK],
                   lhsT=dr_ap(lhsT), rhs=dr_ap(rhs),
                   start=True, stop=True, perf_mode=DR)
            eng = pick_engine(2 * (QBLK - col0), q)
            s_evac(eng, pt[:, k_i, :, col0:QBLK], st[:, :, col0:QBLK])
            if k_i >= 4 * q:   # diagonal tile: zero below-diagonal
                seg = pt[:, k_i, :, col0:col0 + KBLK]
                nc.gpsimd.tensor_mul(seg, seg, mask2)

        yts = {}
        tile_stages_done = {}

        def retire_chunks(q, p, pt, last=False):
            """PV + normalize chunks; transpose/c_proj go to `late` (they
            depend on DVE results of the PV chunks — spacing them a stage
            later avoids PE head-of-line stalls).  For the final stage
            (`last`), everything chains per token-subtile instead so the
            post-last-exp critical path covers one subtile, not four."""
            chunks = []
            late = []
            yas = {}
            yq4s = []

            def pv_group(s, tt):
                if tt == 0:
                    yas[s] = psY.tile([128, 4, KBLK], F32, tag="y", name="ya")
                ya = yas[s]
                h = p * 2 + s
                nkt = 4 * q + tt + 1
                for k_i in range(nkt):
                    MM(
                        ya[:, tt, 0:D + 1],
                        lhsT=pt[:, k_i, s, tt * KBLK:(tt + 1) * KBLK],
                        rhs=vs[:, k_i, h, 0:D + 1],
                        start=(k_i == 0), stop=(k_i == nkt - 1),
                        skip_group_check=True)

            def norm(s):
                # y * (1/rowsum), per token-subtile
                if not yq4s:
                    yq4s.append(yqp.tile([128, 4, 128], F16, tag="yq",
                                         name="yq"))
                ya = yas.pop(s)
                rec = recp.tile([128, 4], F32, tag="rec", name="rec")
                nc.vector.reciprocal_approx_fast(rec, ya[:, :, D:D + 1])
                for tt in range(4):
                    nc.vector.tensor_scalar(
                        yq4s[0][:, tt, s * D:(s + 1) * D], ya[:, tt, 0:D],
                        rec[:, tt:tt + 1], None, op0=ALU.mult)

            def transpose_all():
                # y^T via the DMA XBAR (14ns per 32x32 tile, runs on the
                # mostly-idle DMA engines; frees PE + DVE + a psum bank)
                if q not in yts:
                    yts[q] = ytp.tile([128, NP, QBLK], F16, tag="yt",
                                      name="yt")
                for tt in range(4):
                    nc.sync.dma_start_transpose(
                        yts[q][:, p, tt * KBLK:(tt + 1) * KBLK],
                        yq4s[0][:, tt, :])

            def norm_tt(tt):
                if not yq4s:
                    yq4s.append(yqp.tile([128, 4, 128], F16, tag="yq",
                                         name="yq"))
                rec = recp.tile([128, 2], F32, tag="rec", name="rec")
                for s in range(2):
                    nc.vector.reciprocal_approx_fast(
                        rec[:, s:s + 1], yas[s][:, tt, D:D + 1])
                    nc.vector.tensor_scalar(
                        yq4s[0][:, tt, s * D:(s + 1) * D],
                        yas[s][:, tt, 0:D],
                        rec[:, s:s + 1], None, op0=ALU.mult)

            def transpose_tt(tt):
                if q not in yts:
                    yts[q] = ytp.tile([128, NP, QBLK], F16, tag="yt",
                                      name="yt")
                tp = psQ.tile([128, KBLK], F16, tag="pq", name="tp")
                pe_ord[0] += 2
                nc.tensor.transpose(tp, yq4s[0][:, tt, :], ident)
                nc.vector.tensor_copy(
                    yts[q][:, p, tt * KBLK:(tt + 1) * KBLK], tp)

            if not last:
                for s in range(2):
                    for tt in range(4):
                        chunks.append((lambda s=s, tt=tt: pv_group(s, tt),
                                       (4 * q + tt + 1) * (D + 1) * PE_NS,
                                       f"pv:{q}:{p}:{s}:{tt}"))
                    chunks.append((lambda s=s: norm(s), 0.0,
                                   f"nm:{q}:{p}:{s}"))
                tps = [(transpose_all, 0.0, f"tp:{q}:{p}")]
            else:
                for tt in range(4):
                    for s in range(2):
                        chunks.append((lambda s=s, tt=tt: pv_group(s, tt),
                                       (4 * q + tt + 1) * (D + 1) * PE_NS,
                                       f"pv:{q}:{p}:{s}:{tt}"))
                    chunks.append((lambda tt=tt: norm_tt(tt), 0.0,
                                   f"nm:{q}:{p}:{tt // 3}"))
                    chunks.append((lambda tt=tt: transpose_tt(tt),
                                   128 * PE_NS + 70.0, f"tp:{q}:{p}:{tt}"))
                tps = []

            tile_stages_done[q] = tile_stages_done.get(q, 0) + 1
            late.extend(tps)
            if tile_stages_done[q] == NP:
                obs_local = {}

                def cproj2(tt, ec):
                    if q not in obs_local:
                        obs_local[q] = obp.tile([128, 4, C], F16, tag="ob",
                                                name="ob")
                    po = psQ.tile([128, QBLK], F32, tag="pq", name="po")
                    yt = yts[q]
                    for j in range(NP):
                        MM(
                            po[:, 0:GC],
                            lhsT=yt[:, j, tt * KBLK:(tt + 1) * KBLK],
                            rhs=wp[:, j, ec * GC:(ec + 1) * GC],
                            start=(j == 0), stop=(j == NP - 1))
                    dst = obs_local[q][:, tt, ec * GC:(ec + 1) * GC]
                    if q >= 2:
                        # late tiles: act is idle in the retire tail while
                        # dve is the bottleneck there
                        nc.scalar.copy(dst, po[:, 0:GC])
                    else:
                        nc.vector.tensor_copy(dst, po[:, 0:GC])

                def out_dma(tt):
                    nc.sync.dma_start(out=out_r[q][:, tt, :],
                                      in_=obs_local[q][:, tt, :])
                    if tt == 3:
                        yts.pop(q)
                        obs_local.pop(q)

                cpod = [[] for _ in range(4)]
                for tt in range(4):
                    for ec in range(2):
                        cpod[tt].append(
                            (lambda tt=tt, ec=ec: cproj2(tt, ec),
                             NP * GC * PE_NS, f"cp:{q}:{tt}:{ec}"))
                    cpod[tt].append((lambda tt=tt: out_dma(tt), 0.0,
                                     f"od:{q}:{tt}"))
                if last:
                    # two-step skew: pv(tt) || norm+tp(tt-1) || c_proj(tt-2)
                    # so PE never waits a full DVE chain between subtiles
                    grp = [chunks[4 * tt:4 * tt + 4] for tt in range(4)]
                    newc = []
                    for step in range(6):
                        if step < 4:
                            newc.extend(grp[step][0:2])      # pv pair
                        if 1 <= step <= 4:
                            newc.extend(grp[step - 1][2:4])  # norm, tp
                        if step >= 2:
                            newc.extend(cpod[step - 2])
                    chunks[:] = newc
                else:
                    for tt in range(4):
                        late.extend(cpod[tt])
            return chunks, late

        # ---- main pipelined issue loop ----
        issued = set()

        # Queue A: Q/K projections of ALL tiles (critical path: enables
        # Act's late-tile exp work early).  Queue B: V projections and
        # retire work — drained in the Act-bound phase where PE has slack.
        workA = work
        workB = deque()

        def pop_work():
            src = workA if workA else workB
            chunk, cost, label = src.popleft()
            chunk()
            issued.add(label)
            return cost, label

        def pop_workB():
            chunk, cost, label = workB.popleft()
            chunk()
            issued.add(label)
            return cost, label

        # weight DMAs.  HWDGE is a single shared descriptor generator, so
        # issue order across queues IS landing order: K-hi first (the first
        # qk chain needs it), then x (tile 0, via qk_chunks' dma_x on sync),
        # then Q-hi and the lo parts on the Act queue (whose seq is busy
        # with LoadActFuncSet for the first ~2us anyway).  V and wp go on
        # the gpsimd SWDGE queue — pool is idle this early.
        # weight DMAs: K-hi first (first qk chain), x tile 0 next, then the
        # remaining tensors on the Act queue in need order.  All transfers
        # are [128, 2304]-contiguous on both sides (full DMA rate).
        nc.sync.dma_start(out=w8["k", 0], in_=wqkv_d["wkh"][:, :])
        workA.extend(qk_chunks(0))
        pop_work()   # x^T DMA of tile 0 — next on the sync queue
        nc.scalar.dma_start(out=w8["q", 0], in_=wqkv_d["wqh"][:, :])
        nc.scalar.dma_start(out=w8["k", 1], in_=wqkv_d["wkl"][:, :])
        nc.scalar.dma_start(out=w8["q", 1], in_=wqkv_d["wql"][:, :])
        nc.scalar.dma_start(out=w8["v", 0], in_=wqkv_d["wvh"][:, :])
        nc.scalar.dma_start(out=w8["v", 1], in_=wqkv_d["wvl"][:, :])
        nc.gpsimd.dma_start(out=wp, in_=wp_r)
        # warm the PE p-state while the first DMAs are in flight: dummy
        # matmuls on a const tile keep the array continuously busy so the
        # real Q/K projections start at full clock
        junk = const.tile([128, QBLK], F16)
        nc.vector.memset(junk, 0.0)
        for _ in range(WARMUP):
            jp = psS.tile([128, 2, QBLK], F32, tag="st", name="jp")
            MM(jp[:, 0, :], lhsT=ident, rhs=junk, start=True, stop=True)
        for _ in range(4):   # Q/K of pair 0 eagerly
            pop_work()
        for q in range(1, NQ):
            workA.extend(qk_chunks(q))

        # drain budgets per global S-batch index (measured-stall feedback);
        # records of what was actually drained are kept for the tuner.
        drained_rec = []
        marks = []   # PE event ordinal at the start of each S batch

        stages = [(q, p) for q in range(NQ) for p in range(NP)]
        if HOLDOUT and NQ > 1:
            # hold one small early stage for the end: its S/exp stream hides
            # the last big tile's c_proj, and its own retire tail is short
            stages.remove(HOLDOUT)
            stages.append(HOLDOUT)
        pend_late = []
        b = 0   # global S-batch index
        for i, (q, p) in enumerate(stages):
            if p == 0:
                workB.extend(v_chunks(q))
            # PE-order safety: this stage's Q/K groups must be issued first
            while (f"qk:{q}:{p}:0" not in issued
                   or f"qk:{q}:{p}:1" not in issued):
                pop_work()
            # pt-pool WAR safety: this stage's exp writes reuse the pt slot
            # of stage i-3 — its PV/norm chunks must already be issued, or
            # Act would wait on PE work scheduled after this stage
            if i >= 3:
                oq, op = stages[i - 3]
                while f"nm:{oq}:{op}:1" not in issued:
                    pop_workB()
            nk = 4 * (q + 1)
            pt = ptp.tile([128, nk, 2, QBLK], F16, tag="pt", name="pt")
            for k_i in range(nk):
                budget = gates[b] if gates is not None and b < len(gates) \
                    else DRAIN_NS
                spent = 0.0
                while workA or workB:
                    nxt = (workA or workB)[0][1]
                    if spent + max(nxt, 60.0) > budget + 200.0:
                        break
                    c, lab = pop_work()
                    spent += max(c, 60.0)
                drained_rec.append(spent)
                marks.append(pe_ord[0])
                s_batch(q, p, k_i, pt)
                b += 1
            if i == len(stages) - 1:
                # final stage: prior pair's transposes must precede its
                # per-subtile c_proj chains in issue order
                chunks, late = retire_chunks(q, p, pt, last=True)
                workB.extend(pend_late)
                workB.extend(chunks)
            else:
                chunks, late = retire_chunks(q, p, pt)
                workB.extend(chunks)
                workB.extend(pend_late)
            pend_late = late
        workB.extend(pend_late)
        while workA or workB:
            pop_work()

        build_nc.last_drained = drained_rec
        build_nc.last_marks = marks

    nc.compile()
    return nc


def make_in_map(x_b, w_attn, w_proj, g):
    """Per-core input arrays for batch slice x_b and head-group g."""
    import ml_dtypes
    E4 = ml_dtypes.float8_e4m3fn

    def shuf(w):
        # [768, 384] -> [r=128, (c2 j m)=2304]: row r holds the weights
        # for contraction rows c2*256 + j*128 + r, matching the on-device
        # DoubleRow tile layout with a fully contiguous DMA
        return np.ascontiguousarray(
            w.reshape(3, 2, 128, GC).transpose(2, 0, 1, 3).reshape(128, -1))

    sl = slice(g * GC, (g + 1) * GC)
    out = {}
    for nm, w in (("wq", w_attn[:, 0 * C:1 * C][:, sl]),
                  ("wk", w_attn[:, 1 * C:2 * C][:, sl]),
                  ("wv", w_attn[:, 2 * C:3 * C][:, sl])):
        ww = np.ascontiguousarray(w).astype(np.float32) * 32.0
        hi = ww.astype(E4)
        lo = (ww - hi.astype(np.float32)).astype(E4)
        out[f"w{nm[1]}h"] = shuf(hi)
        out[f"w{nm[1]}l"] = shuf(lo)
    xt = np.ascontiguousarray(x_b.T).astype(np.float32)
    xth = xt.astype(E4)
    out["xth"] = xth
    out["xtl"] = (xt - xth.astype(np.float32)).astype(E4)
    out["wp"] = np.ascontiguousarray(w_proj[sl, :]).astype(np.float16)
    return out


_NC_CACHE = {}


def _get_nc(T):
    if T not in _NC_CACHE:
        _NC_CACHE[T] = build_nc(T)
    return _NC_CACHE[T]


def kernel(x, w_attn, b_attn, w_proj, b_proj, _trace=False):
    from concourse.bass_utils import run_bass_kernel_spmd

    x = np.asarray(x, dtype=np.float32)
    w_attn = np.asarray(w_attn, dtype=np.float32)
    b_attn = np.asarray(b_attn, dtype=np.float32)
    w_proj = np.asarray(w_proj, dtype=np.float32)
    b_proj = np.asarray(b_proj, dtype=np.float32)
    B, T, _ = x.shape

    assert not np.any(b_attn[0:2 * C] != 0.0), \
        "nonzero q/k bias not supported by this kernel"

    nc = _get_nc(T)
    in_maps = []
    for b in range(B):
        for g in range(2):
            in_maps.append(make_in_map(x[b], w_attn, w_proj, g))
    res = run_bass_kernel_spmd(nc, in_maps, core_ids=list(range(2 * B)),
                               trace=_trace)
    outs = [np.asarray(r["out"], dtype=np.float32) for r in res.results]
    # softmax rows sum to 1, so the V-bias contribution is exactly
    # bv @ w_proj added to every token (not computed on device).
    bias_row = b_proj + b_attn[2 * C:3 * C] @ w_proj
    out = np.empty((B, T, C), dtype=np.float32)
    for b in range(B):
        out[b] = outs[2 * b] + outs[2 * b + 1] + bias_row[None, :]
    if _trace:
        kernel.last_result = res
    return out



# revision 71
# speedup vs baseline: 1.0017x; 1.0017x over previous
"""Causal self-attention (B=4 T=2048 C=768 H=12) on 8 trn2 cores — v3.

Sharding: core = (batch b, head-group g), g in {0,1} covering 6 heads
(3 pairs).  Host sums the two partial c_proj outputs per batch and adds
the exact bias row (b_proj + bv @ w_proj; softmax rows sum to 1).

v3 design (vs v2, 140850 -> 130486 ns):
  * hi/lo fp8 QKV projections: host splits x and 32*w_attn into e4m3
    hi + residual-lo (subnormal) pairs; three 256-contraction DoubleRow
    chains (hi.hi + hi.lo + lo.hi) in one psum accumulation run at 2x
    the fp16 rate with ~0.14% error.  1/32 unapplied in the evacuations.
  * S^T = K^T.Q per (pair, head) via fp8e4m3 DoubleRow matmuls at 0.5
    cycles/row (contraction-64 as 2 broadcast slots; exp scale 0.0625
    absorbs the doubling).
  * S evacuation split across engines (pool is barred from PSUM): act
    does true exp for ~58% of elements, dve does a 1-op fp16-bitcast
    schraudolph exp (tensor_scalar mult/add into an int16-bitcast view,
    1.8% RMS on its share; tile-scattered so end-to-end cost is ~5e-4).
    Per-q act shares lean higher for late tiles (dve carries retire).
  * causal masks on the pool engine (SBUF-only); y normalization via
    reciprocal + per-subtile tensor_scalar on dve; y^T via XBAR
    transpose-DMAs on the sync queue (PE/DVE freed); c_proj evacuation
    on dve early / act late (act idles in the tail).
  * chunk-queue software pipelining with a small (160ns) per-S-batch
    drain budget; weight/x DMA issue order tuned against the single
    serializing DMA_ENGINES device (K-hi, x0, Q-hi, los, V, wp).
"""

from collections import deque
from contextlib import ExitStack

import numpy as np

import concourse.bass as bass
import concourse.mybir as mybir
import concourse.tile as tile
from concourse import bacc
from concourse.masks import make_upper_triangular, make_identity

AF = mybir.ActivationFunctionType
ALU = mybir.AluOpType
F32 = mybir.dt.float32
F16 = mybir.dt.float16
F8 = mybir.dt.float8e4
I16 = mybir.dt.int16
DR = mybir.MatmulPerfMode.DoubleRow

# fp16-bitcast schraudolph exp approximation: exp(s*scale) ~=
# bitcast16(int16(A_SCHR*s + B_SCHR)).  C=-0.0575 minimizes RMS rel err
# (1.78%); +0.5 turns the trunc-on-int-store into round-to-nearest.
SCHR_SCALE = 0.0625          # fp8-DR doubled psum units
A_SCHR = float(np.float32(1024.0 * 1.4426950408889634 * SCHR_SCALE))
B_SCHR = float(np.float32(15 * 1024 + 1024.0 * (-0.0575) + 0.5))

# engine shares for the S-stream evacuation (fraction of elements).
# pool cannot touch PSUM (hard BIR rule), so the split is act (true exp,
# 0.833/elem) vs dve (schraudolph, 1.04/elem on fp32 psum).
SHARE = {"act": 0.60, "dve": 0.40}
# per-q act-share overrides: late (big) stages run while dve also carries
# the retire stream, so they lean harder on act
SHARE_BY_Q = {0: 0.55, 1: 0.60, 2: 0.60, 3: 0.62}
ALTERNATE = False   # strict act/dve alternation for S evacuation

C = 768          # model dim
D = 64           # head dim
HG = 6           # heads per core
NP = 3           # head pairs per core
GC = HG * D      # 384 group channels
CT = C // 128    # 6 contraction tiles
QBLK = 512       # query tile (psum bank)
KBLK = 128       # key tile

S_FP8 = True     # fp8e4m3 DoubleRow for the S matmul (else fp16)
WARMUP = 12      # PE p-state warmup matmuls at startup
HOLDOUT = None   # stage held to the end to shorten the tail (None = off)
DRAIN_NS = 160.0  # default per-S-batch PE-work drain budget (gates override)


def build_nc(T=2048, s_fp8=S_FP8, gates=None):
    NQ = T // QBLK
    NK = T // KBLK
    nc = bacc.Bacc(None)

    # x and w_attn arrive as hi/lo fp8 pairs: x ~= xth + xtl (lo holds the
    # quantization residual, subnormal-heavy), w_attn scaled by 32 so its
    # hi part uses the e4m3 normal range; the 1/32 is unapplied in the
    # q/k/v psum evacuations.  Projections run as three fp8 DoubleRow
    # chains (hi.hi + hi.lo + lo.hi) at 2x the fp16 matmul rate.
    xth_d = nc.dram_tensor("xth", [C, T], F8, kind="ExternalInput")
    xtl_d = nc.dram_tensor("xtl", [C, T], F8, kind="ExternalInput")
    # per-projection weights, host-PRE-SHUFFLED to [r=128, (c2 j m)] so
    # both DMA sides are 2304B-contiguous (full-rate; a 384B-row slice
    # transfer pays the <512B half-rate penalty)
    wqkv_d = {}
    for nm in ("wqh", "wql", "wkh", "wkl", "wvh", "wvl"):
        wqkv_d[nm] = nc.dram_tensor(nm, [128, 6 * GC], F8,
                                    kind="ExternalInput")
    wp_d = nc.dram_tensor("wp", [GC, C], F16, kind="ExternalInput")
    out_d = nc.dram_tensor("out", [T, C], F16, kind="ExternalOutput")
    W_UNSCALE = 1.0 / 32.0

    qk_dt = F8 if s_fp8 else F16
    exp_scale = 0.0625 if s_fp8 else 0.125

    with ExitStack() as ctx:
        tc = ctx.enter_context(tile.TileContext(nc))
        const = ctx.enter_context(tc.tile_pool(name="const", bufs=1))
        big = ctx.enter_context(tc.tile_pool(name="big", bufs=1))
        xtp = ctx.enter_context(tc.tile_pool(name="xtp", bufs=4))
        ptp = ctx.enter_context(tc.tile_pool(name="ptp", bufs=3))
        yqp = ctx.enter_context(tc.tile_pool(name="yqp", bufs=3))
        recp = ctx.enter_context(tc.tile_pool(name="recp", bufs=3))
        ytp = ctx.enter_context(tc.tile_pool(name="ytp", bufs=4))
        obp = ctx.enter_context(tc.tile_pool(name="obp", bufs=2))
        psS = ctx.enter_context(tc.tile_pool(name="psS", bufs=2, space="PSUM"))
        psY = ctx.enter_context(tc.tile_pool(name="psY", bufs=2, space="PSUM"))
        psQ = ctx.enter_context(tc.tile_pool(name="psQ", bufs=2, space="PSUM"))

        # constants.  ident first: the PE warmup matmuls depend on it, and
        # everything later on the pool queue (SWDGE descriptor generation,
        # mask builds) would delay it by ~3us.
        ident = const.tile([128, 128], F16)
        make_identity(nc, ident)
        mask2 = const.tile([128, 2, KBLK], F16)   # causal keep-mask, 2 heads
        make_upper_triangular(nc, mask2[:, 0, :], val=1.0, diag=True)
        make_upper_triangular(nc, mask2[:, 1, :], val=1.0, diag=True)

        # persistent.  weight/x layout for 256-contraction DoubleRow:
        # [r=128, ct2 in 3, j in 2, cols]; contraction c = ct2*256+j*128+r.
        CT2 = 3
        w8 = {}   # (proj, hl) -> [128, CT2, 2, GC] tile
        for proj in "qkv":
            for hl in (0, 1):
                w8[proj, hl] = big.tile([128, CT2, 2, GC], F8,
                                        name=f"w8{proj}{hl}")
        wp = big.tile([128, NP, C], F16)
        kt8 = big.tile([128, NP, T], qk_dt)
        qt8 = big.tile([128, NQ, NP, QBLK], qk_dt)
        # V with trailing ones column per (ktile, head): [k, 66] rows
        vs = big.tile([128, NK, HG, D + 2], F16)
        nc.gpsimd.memset(vs[:, :, :, D:D + 1], 1.0)

        xt_r = {0: xth_d[:, :].rearrange("(c2 j r) t -> r c2 j t",
                                         r=128, j=2),
                1: xtl_d[:, :].rearrange("(c2 j r) t -> r c2 j t",
                                         r=128, j=2)}
        # (xw)-chain list: (x hi/lo, w hi/lo)
        CHAINS = ((0, 0), (0, 1), (1, 0))
        wp_r = wp_d[:, :].rearrange("(p r) e -> r p e", r=128)
        out_r = out_d[:, :].rearrange("(q tt r) e -> q r tt e", tt=QBLK // KBLK,
                                      r=128)

        def dr_ap(ap):
            """[64, N] fp8 AP -> [64, 2, N] stride-0 DoubleRow operand."""
            return ap.unsqueeze(1).broadcast_to(
                [ap.shape[0], 2] + list(ap.shape[1:]))

        # ---- chunk helpers (each chunk = (callable, pe_ns), issued later) --
        PE_NS = 1.0 / 2.4   # ns per PE cycle at full speed
        work = deque()

        pe_ord = [0]   # PE event ordinal (Ldweights+Matmult pairs)

        def MM(*a, **k):
            pe_ord[0] += 2
            return nc.tensor.matmul(*a, **k)

        xtqs = {}

        def qk_chunks(q):
            """DMA + Q/K projection groups for tile q (critical early path)."""
            qs = q * QBLK

            def dma_x():
                xtq = xtp.tile([128, 2, CT2, 2, QBLK], F8, tag="xtq",
                               name="xtq")
                xtqs[q] = xtq
                for hl in (0, 1):   # hi first: half-0 chains need only hi
                    nc.sync.dma_start(out=xtq[:, hl, :, :, :],
                                      in_=xt_r[hl][:, :, :, qs:qs + QBLK])

            chunks = [(dma_x, 0.0, f"dx:{q}")]

            pss = {}

            def qk_part(p, which, half):
                if half == 0:
                    pss[(p, which)] = psQ.tile([128, QBLK], F32, tag="pq",
                                               name="pqk")
                ps = pss[(p, which)]
                proj = "q" if which == 0 else "k"
                # half 0: the hi.hi chain; half 1: the two cross chains
                chains = CHAINS[0:1] if half == 0 else CHAINS[1:3]
                for ci, (xs, ws) in enumerate(chains):
                    for ct2 in range(CT2):
                        first = half == 0 and ci == 0 and ct2 == 0
                        last = half == 1 and ci == len(chains) - 1 \
                            and ct2 == CT2 - 1
                        MM(ps, lhsT=w8[proj, ws][:, ct2, :,
                                                 p * 128:(p + 1) * 128],
                           rhs=xtqs[q][:, xs, ct2, :, :],
                           start=first, stop=last, perf_mode=DR)
                if half == 1:
                    del pss[(p, which)]
                    if which == 0:
                        nc.scalar.mul(qt8[:, q, p, :], ps, W_UNSCALE)
                    else:
                        nc.scalar.mul(kt8[:, p, qs:qs + QBLK], ps, W_UNSCALE)

            # half-0 (hi.hi) chains first within each pair: they need only
            # the hi tensors, which DMA-land first
            for p in range(NP):
                for half in (0, 1):
                    for which in (1, 0):   # K first
                        chunks.append(
                            (lambda p=p, w=which, h=half: qk_part(p, w, h),
                             (3 if half == 0 else 6) * 256 * PE_NS,
                             f"qk:{q}:{p}:{which}" if half else
                             f"qka:{q}:{p}:{which}"))
            return chunks

        def v_chunks(q):
            """V projection groups for tile q (needed by PV, not by S/exp)."""

            pss = {}

            def v_part(kl, half):
                k_i = 4 * q + kl
                if half == 0:
                    pss[kl] = psQ.tile([128, QBLK], F32, tag="pq", name="pv")
                ps = pss[kl]
                chains = CHAINS[0:1] if half == 0 else CHAINS[1:3]
                for ci, (xs, ws) in enumerate(chains):
                    for ct2 in range(CT2):
                        first = half == 0 and ci == 0 and ct2 == 0
                        last = half == 1 and ci == len(chains) - 1 \
                            and ct2 == CT2 - 1
                        MM(ps[:, 0:GC],
                           lhsT=xtqs[q][:, xs, ct2, :,
                                        kl * KBLK:(kl + 1) * KBLK],
                           rhs=w8["v", ws][:, ct2, :, :],
                           start=first, stop=last, perf_mode=DR)
                if half == 1:
                    del pss[kl]
                    nc.scalar.mul(
                        vs[:, k_i, :, 0:D],
                        ps[:, 0:GC].rearrange("r (h d) -> r h d", d=D),
                        W_UNSCALE)

            return [(lambda kl=kl, h=h: v_part(kl, h),
                     (3 if h == 0 else 6) * (GC // 2) * PE_NS,
                     f"v:{q}:{kl}" if h else f"va:{q}:{kl}")
                    for kl in range(4) for h in (0, 1)]

        # engine assignment for S-stream evacuation.  strict alternation:
        # consecutive evacuations on the same engine serialize (~1.1us
        # each) while alternating ones overlap, halving the S-stream
        # period.  ALT_PATTERN cycles act/dve; deficit mode weights by
        # SHARE instead.
        eng_credit = {"act": 0.0, "dve": 0.0}
        alt_state = [0]

        def pick_engine(elems, q=None):
            if ALTERNATE:
                e = ("act", "dve")[alt_state[0] % 2]
                alt_state[0] += 1
                return e
            a = SHARE_BY_Q.get(q, SHARE["act"])
            shares = {"act": a, "dve": 1.0 - a}
            for e in eng_credit:
                eng_credit[e] += shares[e] * elems
            best = max(eng_credit, key=lambda e: eng_credit[e])
            eng_credit[best] -= elems
            return best

        def s_evac(eng, dst, src):
            if eng == "act":
                nc.scalar.activation(dst, src, AF.Exp, scale=exp_scale)
            else:
                nc.vector.tensor_scalar(dst.bitcast(I16), src,
                                        A_SCHR, B_SCHR,
                                        op0=ALU.mult, op1=ALU.add)

        def s_batch(q, p, k_i, pt):
            """S^T (both heads) + exp + mask for one k-tile."""
            col0 = max(k_i - 4 * q, 0) * KBLK
            st = psS.tile([128, 2, QBLK], F32, tag="st", name="st")
            for s in range(2):
                hoff = D * s
                lhsT = kt8[hoff:hoff + D, p, k_i * KBLK:(k_i + 1) * KBLK]
                rhs = qt8[hoff:hoff + D, q, p, col0:QBLK]
                MM(st[:, s, col0:QBLK],
                   lhsT=dr_ap(lhsT), rhs=dr_ap(rhs),
                   start=True, stop=True, perf_mode=DR)
            if q == 3 and p == 2 and k_i >= 14:
                # program-final batches: act (the faster evacuator) so the
                # retire tail starts as early as possible
                eng = "act"
            else:
                eng = pick_engine(2 * (QBLK - col0), q)
            s_evac(eng, pt[:, k_i, :, col0:QBLK], st[:, :, col0:QBLK])
            if k_i >= 4 * q:   # diagonal tile: zero below-diagonal
                seg = pt[:, k_i, :, col0:col0 + KBLK]
                nc.gpsimd.tensor_mul(seg, seg, mask2)

        yts = {}
        tile_stages_done = {}

        def retire_chunks(q, p, pt, last=False):
            """PV + normalize chunks; transpose/c_proj go to `late` (they
            depend on DVE results of the PV chunks — spacing them a stage
            later avoids PE head-of-line stalls).  For the final stage
            (`last`), everything chains per token-subtile instead so the
            post-last-exp critical path covers one subtile, not four."""
            chunks = []
            late = []
            yas = {}
            yq4s = []

            def pv_group(s, tt):
                if tt == 0:
                    yas[s] = psY.tile([128, 4, KBLK], F32, tag="y", name="ya")
                ya = yas[s]
                h = p * 2 + s
                nkt = 4 * q + tt + 1
                for k_i in range(nkt):
                    MM(
                        ya[:, tt, 0:D + 1],
                        lhsT=pt[:, k_i, s, tt * KBLK:(tt + 1) * KBLK],
                        rhs=vs[:, k_i, h, 0:D + 1],
                        start=(k_i == 0), stop=(k_i == nkt - 1),
                        skip_group_check=True)

            def norm(s):
                # y * (1/rowsum), per token-subtile
                if not yq4s:
                    yq4s.append(yqp.tile([128, 4, 128], F16, tag="yq",
                                         name="yq"))
                ya = yas.pop(s)
                rec = recp.tile([128, 4], F32, tag="rec", name="rec")
                nc.vector.reciprocal_approx_fast(rec, ya[:, :, D:D + 1])
                for tt in range(4):
                    nc.vector.tensor_scalar(
                        yq4s[0][:, tt, s * D:(s + 1) * D], ya[:, tt, 0:D],
                        rec[:, tt:tt + 1], None, op0=ALU.mult)

            def transpose_all():
                # y^T via the DMA XBAR (14ns per 32x32 tile, runs on the
                # mostly-idle DMA engines; frees PE + DVE + a psum bank)
                if q not in yts:
                    yts[q] = ytp.tile([128, NP, QBLK], F16, tag="yt",
                                      name="yt")
                for tt in range(4):
                    nc.sync.dma_start_transpose(
                        yts[q][:, p, tt * KBLK:(tt + 1) * KBLK],
                        yq4s[0][:, tt, :])

            def norm_tt(tt):
                if not yq4s:
                    yq4s.append(yqp.tile([128, 4, 128], F16, tag="yq",
                                         name="yq"))
                rec = recp.tile([128, 2], F32, tag="rec", name="rec")
                for s in range(2):
                    nc.vector.reciprocal_approx_fast(
                        rec[:, s:s + 1], yas[s][:, tt, D:D + 1])
                    nc.vector.tensor_scalar(
                        yq4s[0][:, tt, s * D:(s + 1) * D],
                        yas[s][:, tt, 0:D],
                        rec[:, s:s + 1], None, op0=ALU.mult)

            def transpose_tt(tt):
                if q not in yts:
                    yts[q] = ytp.tile([128, NP, QBLK], F16, tag="yt",
                                      name="yt")
                tp = psQ.tile([128, KBLK], F16, tag="pq", name="tp")
                pe_ord[0] += 2
                nc.tensor.transpose(tp, yq4s[0][:, tt, :], ident)
                nc.vector.tensor_copy(
                    yts[q][:, p, tt * KBLK:(tt + 1) * KBLK], tp)

            if not last:
                for s in range(2):
                    for tt in range(4):
                        chunks.append((lambda s=s, tt=tt: pv_group(s, tt),
                                       (4 * q + tt + 1) * (D + 1) * PE_NS,
                                       f"pv:{q}:{p}:{s}:{tt}"))
                    chunks.append((lambda s=s: norm(s), 0.0,
                                   f"nm:{q}:{p}:{s}"))
                tps = [(transpose_all, 0.0, f"tp:{q}:{p}")]
            else:
                for tt in range(4):
                    for s in range(2):
                        chunks.append((lambda s=s, tt=tt: pv_group(s, tt),
                                       (4 * q + tt + 1) * (D + 1) * PE_NS,
                                       f"pv:{q}:{p}:{s}:{tt}"))
                    chunks.append((lambda tt=tt: norm_tt(tt), 0.0,
                                   f"nm:{q}:{p}:{tt // 3}"))
                    chunks.append((lambda tt=tt: transpose_tt(tt),
                                   128 * PE_NS + 70.0, f"tp:{q}:{p}:{tt}"))
                tps = []

            tile_stages_done[q] = tile_stages_done.get(q, 0) + 1
            late.extend(tps)
            if tile_stages_done[q] == NP:
                obs_local = {}

                def cproj2(tt, ec):
                    if q not in obs_local:
                        obs_local[q] = obp.tile([128, 4, C], F16, tag="ob",
                                                name="ob")
                    po = psQ.tile([128, QBLK], F32, tag="pq", name="po")
                    yt = yts[q]
                    for j in range(NP):
                        MM(
                            po[:, 0:GC],
                            lhsT=yt[:, j, tt * KBLK:(tt + 1) * KBLK],
                            rhs=wp[:, j, ec * GC:(ec + 1) * GC],
                            start=(j == 0), stop=(j == NP - 1))
                    dst = obs_local[q][:, tt, ec * GC:(ec + 1) * GC]
                    if q >= 2:
                        # late tiles: act is idle in the retire tail while
                        # dve is the bottleneck there
                        nc.scalar.copy(dst, po[:, 0:GC])
                    else:
                        nc.vector.tensor_copy(dst, po[:, 0:GC])

                def out_dma(tt):
                    nc.sync.dma_start(out=out_r[q][:, tt, :],
                                      in_=obs_local[q][:, tt, :])
                    if tt == 3:
                        yts.pop(q)
                        obs_local.pop(q)

                cpod = [[] for _ in range(4)]
                for tt in range(4):
                    for ec in range(2):
                        cpod[tt].append(
                            (lambda tt=tt, ec=ec: cproj2(tt, ec),
                             NP * GC * PE_NS, f"cp:{q}:{tt}:{ec}"))
                    cpod[tt].append((lambda tt=tt: out_dma(tt), 0.0,
                                     f"od:{q}:{tt}"))
                if last:
                    # two-step skew: pv(tt) || norm+tp(tt-1) || c_proj(tt-2)
                    # so PE never waits a full DVE chain between subtiles
                    grp = [chunks[4 * tt:4 * tt + 4] for tt in range(4)]
                    newc = []
                    for step in range(6):
                        if step < 4:
                            newc.extend(grp[step][0:2])      # pv pair
                        if 1 <= step <= 4:
                            newc.extend(grp[step - 1][2:4])  # norm, tp
                        if step >= 2:
                            newc.extend(cpod[step - 2])
                    chunks[:] = newc
                else:
                    for tt in range(4):
                        late.extend(cpod[tt])
            return chunks, late

        # ---- main pipelined issue loop ----
        issued = set()

        # Queue A: Q/K projections of ALL tiles (critical path: enables
        # Act's late-tile exp work early).  Queue B: V projections and
        # retire work — drained in the Act-bound phase where PE has slack.
        workA = work
        workB = deque()

        def pop_work():
            src = workA if workA else workB
            chunk, cost, label = src.popleft()
            chunk()
            issued.add(label)
            return cost, label

        def pop_workB():
            chunk, cost, label = workB.popleft()
            chunk()
            issued.add(label)
            return cost, label

        # weight DMAs.  HWDGE is a single shared descriptor generator, so
        # issue order across queues IS landing order: K-hi first (the first
        # qk chain needs it), then x (tile 0, via qk_chunks' dma_x on sync),
        # then Q-hi and the lo parts on the Act queue (whose seq is busy
        # with LoadActFuncSet for the first ~2us anyway).  V and wp go on
        # the gpsimd SWDGE queue — pool is idle this early.
        # weight DMAs: K-hi first (first qk chain), x tile 0 next, then the
        # remaining tensors on the Act queue in need order.  All transfers
        # are [128, 2304]-contiguous on both sides (full DMA rate).
        nc.sync.dma_start(out=w8["k", 0], in_=wqkv_d["wkh"][:, :])
        workA.extend(qk_chunks(0))
        pop_work()   # x^T DMA of tile 0 — next on the sync queue
        nc.scalar.dma_start(out=w8["q", 0], in_=wqkv_d["wqh"][:, :])
        nc.scalar.dma_start(out=w8["k", 1], in_=wqkv_d["wkl"][:, :])
        nc.scalar.dma_start(out=w8["q", 1], in_=wqkv_d["wql"][:, :])
        nc.scalar.dma_start(out=w8["v", 0], in_=wqkv_d["wvh"][:, :])
        nc.scalar.dma_start(out=w8["v", 1], in_=wqkv_d["wvl"][:, :])
        nc.gpsimd.dma_start(out=wp, in_=wp_r)
        # warm the PE p-state while the first DMAs are in flight: dummy
        # matmuls on a const tile keep the array continuously busy so the
        # real Q/K projections start at full clock
        junk = const.tile([128, QBLK], F16)
        nc.vector.memset(junk, 0.0)
        for _ in range(WARMUP):
            jp = psS.tile([128, 2, QBLK], F32, tag="st", name="jp")
            MM(jp[:, 0, :], lhsT=ident, rhs=junk, start=True, stop=True)
        for _ in range(4):   # Q/K of pair 0 eagerly
            pop_work()
        for q in range(1, NQ):
            workA.extend(qk_chunks(q))

        # drain budgets per global S-batch index (measured-stall feedback);
        # records of what was actually drained are kept for the tuner.
        drained_rec = []
        marks = []   # PE event ordinal at the start of each S batch

        stages = [(q, p) for q in range(NQ) for p in range(NP)]
        if HOLDOUT and NQ > 1:
            # hold one small early stage for the end: its S/exp stream hides
            # the last big tile's c_proj, and its own retire tail is short
            stages.remove(HOLDOUT)
            stages.append(HOLDOUT)
        pend_late = []
        b = 0   # global S-batch index
        for i, (q, p) in enumerate(stages):
            if p == 0:
                workB.extend(v_chunks(q))
            # PE-order safety: this stage's Q/K groups must be issued first
            while (f"qk:{q}:{p}:0" not in issued
                   or f"qk:{q}:{p}:1" not in issued):
                pop_work()
            # pt-pool WAR safety: this stage's exp writes reuse the pt slot
            # of stage i-3 — its PV/norm chunks must already be issued, or
            # Act would wait on PE work scheduled after this stage
            if i >= 3:
                oq, op = stages[i - 3]
                while f"nm:{oq}:{op}:1" not in issued:
                    pop_workB()
            nk = 4 * (q + 1)
            pt = ptp.tile([128, nk, 2, QBLK], F16, tag="pt", name="pt")
            for k_i in range(nk):
                budget = gates[b] if gates is not None and b < len(gates) \
                    else DRAIN_NS
                spent = 0.0
                while workA or workB:
                    nxt = (workA or workB)[0][1]
                    if spent + max(nxt, 60.0) > budget + 200.0:
                        break
                    c, lab = pop_work()
                    spent += max(c, 60.0)
                drained_rec.append(spent)
                marks.append(pe_ord[0])
                s_batch(q, p, k_i, pt)
                b += 1
            if i == len(stages) - 1:
                # final stage: prior pair's transposes must precede its
                # per-subtile c_proj chains in issue order
                chunks, late = retire_chunks(q, p, pt, last=True)
                workB.extend(pend_late)
                workB.extend(chunks)
            else:
                chunks, late = retire_chunks(q, p, pt)
                workB.extend(chunks)
                workB.extend(pend_late)
            pend_late = late
        workB.extend(pend_late)
        while workA or workB:
            pop_work()

        build_nc.last_drained = drained_rec
        build_nc.last_marks = marks

    nc.compile()
    return nc


def make_in_map(x_b, w_attn, w_proj, g):
    """Per-core input arrays for batch slice x_b and head-group g."""
    import ml_dtypes
    E4 = ml_dtypes.float8_e4m3fn

    def shuf(w):
        # [768, 384] -> [r=128, (c2 j m)=2304]: row r holds the weights
        # for contraction rows c2*256 + j*128 + r, matching the on-device
        # DoubleRow tile layout with a fully contiguous DMA
        return np.ascontiguousarray(
            w.reshape(3, 2, 128, GC).transpose(2, 0, 1, 3).reshape(128, -1))

    sl = slice(g * GC, (g + 1) * GC)
    out = {}
    for nm, w in (("wq", w_attn[:, 0 * C:1 * C][:, sl]),
                  ("wk", w_attn[:, 1 * C:2 * C][:, sl]),
                  ("wv", w_attn[:, 2 * C:3 * C][:, sl])):
        ww = np.ascontiguousarray(w).astype(np.float32) * 32.0
        hi = ww.astype(E4)
        lo = (ww - hi.astype(np.float32)).astype(E4)
        out[f"w{nm[1]}h"] = shuf(hi)
        out[f"w{nm[1]}l"] = shuf(lo)
    xt = np.ascontiguousarray(x_b.T).astype(np.float32)
    xth = xt.astype(E4)
    out["xth"] = xth
    out["xtl"] = (xt - xth.astype(np.float32)).astype(E4)
    out["wp"] = np.ascontiguousarray(w_proj[sl, :]).astype(np.float16)
    return out


_NC_CACHE = {}


def _get_nc(T):
    if T not in _NC_CACHE:
        _NC_CACHE[T] = build_nc(T)
    return _NC_CACHE[T]


def kernel(x, w_attn, b_attn, w_proj, b_proj, _trace=False):
    from concourse.bass_utils import run_bass_kernel_spmd

    x = np.asarray(x, dtype=np.float32)
    w_attn = np.asarray(w_attn, dtype=np.float32)
    b_attn = np.asarray(b_attn, dtype=np.float32)
    w_proj = np.asarray(w_proj, dtype=np.float32)
    b_proj = np.asarray(b_proj, dtype=np.float32)
    B, T, _ = x.shape

    assert not np.any(b_attn[0:2 * C] != 0.0), \
        "nonzero q/k bias not supported by this kernel"

    nc = _get_nc(T)
    in_maps = []
    for b in range(B):
        for g in range(2):
            in_maps.append(make_in_map(x[b], w_attn, w_proj, g))
    res = run_bass_kernel_spmd(nc, in_maps, core_ids=list(range(2 * B)),
                               trace=_trace)
    outs = [np.asarray(r["out"], dtype=np.float32) for r in res.results]
    # softmax rows sum to 1, so the V-bias contribution is exactly
    # bv @ w_proj added to every token (not computed on device).
    bias_row = b_proj + b_attn[2 * C:3 * C] @ w_proj
    out = np.empty((B, T, C), dtype=np.float32)
    for b in range(B):
        out[b] = outs[2 * b] + outs[2 * b + 1] + bias_row[None, :]
    if _trace:
        kernel.last_result = res
    return out



# revision 74
# speedup vs baseline: 1.0023x; 1.0006x over previous
"""Causal self-attention (B=4 T=2048 C=768 H=12) on 8 trn2 cores — v3.

Sharding: core = (batch b, head-group g), g in {0,1} covering 6 heads
(3 pairs).  Host sums the two partial c_proj outputs per batch and adds
the exact bias row (b_proj + bv @ w_proj; softmax rows sum to 1).

v3 design (vs v2, 140850 -> 130486 ns):
  * hi/lo fp8 QKV projections: host splits x and 32*w_attn into e4m3
    hi + residual-lo (subnormal) pairs; three 256-contraction DoubleRow
    chains (hi.hi + hi.lo + lo.hi) in one psum accumulation run at 2x
    the fp16 rate with ~0.14% error.  1/32 unapplied in the evacuations.
  * S^T = K^T.Q per (pair, head) via fp8e4m3 DoubleRow matmuls at 0.5
    cycles/row (contraction-64 as 2 broadcast slots; exp scale 0.0625
    absorbs the doubling).
  * S evacuation split across engines (pool is barred from PSUM): act
    does true exp for ~58% of elements, dve does a 1-op fp16-bitcast
    schraudolph exp (tensor_scalar mult/add into an int16-bitcast view,
    1.8% RMS on its share; tile-scattered so end-to-end cost is ~5e-4).
    Per-q act shares lean higher for late tiles (dve carries retire).
  * causal masks on the pool engine (SBUF-only); y normalization via
    reciprocal + per-subtile tensor_scalar on dve; y^T via XBAR
    transpose-DMAs on the sync queue (PE/DVE freed); c_proj evacuation
    on dve early / act late (act idles in the tail).
  * chunk-queue software pipelining with a small (160ns) per-S-batch
    drain budget; weight/x DMA issue order tuned against the single
    serializing DMA_ENGINES device (K-hi, x0, Q-hi, los, V, wp).
"""

from collections import deque
from contextlib import ExitStack

import numpy as np

import concourse.bass as bass
import concourse.mybir as mybir
import concourse.tile as tile
from concourse import bacc
from concourse.masks import make_upper_triangular, make_identity

AF = mybir.ActivationFunctionType
ALU = mybir.AluOpType
F32 = mybir.dt.float32
F16 = mybir.dt.float16
F8 = mybir.dt.float8e4
I16 = mybir.dt.int16
DR = mybir.MatmulPerfMode.DoubleRow

# fp16-bitcast schraudolph exp approximation: exp(s*scale) ~=
# bitcast16(int16(A_SCHR*s + B_SCHR)).  C=-0.0575 minimizes RMS rel err
# (1.78%); +0.5 turns the trunc-on-int-store into round-to-nearest.
SCHR_SCALE = 0.0625          # fp8-DR doubled psum units
A_SCHR = float(np.float32(1024.0 * 1.4426950408889634 * SCHR_SCALE))
B_SCHR = float(np.float32(15 * 1024 + 1024.0 * (-0.0575) + 0.5))

# engine shares for the S-stream evacuation (fraction of elements).
# pool cannot touch PSUM (hard BIR rule), so the split is act (true exp,
# 0.833/elem) vs dve (schraudolph, 1.04/elem on fp32 psum).
SHARE = {"act": 0.60, "dve": 0.40}
# per-q act-share overrides: late (big) stages run while dve also carries
# the retire stream, so they lean harder on act
SHARE_BY_Q = {0: 0.55, 1: 0.60, 2: 0.60, 3: 0.62}
ALTERNATE = False   # strict act/dve alternation for S evacuation

C = 768          # model dim
D = 64           # head dim
HG = 6           # heads per core
NP = 3           # head pairs per core
GC = HG * D      # 384 group channels
CT = C // 128    # 6 contraction tiles
QBLK = 512       # query tile (psum bank)
KBLK = 128       # key tile

S_FP8 = True     # fp8e4m3 DoubleRow for the S matmul (else fp16)
WARMUP = 12      # PE p-state warmup matmuls at startup
HOLDOUT = None   # stage held to the end to shorten the tail (None = off)
DRAIN_NS = 160.0  # default per-S-batch PE-work drain budget (gates override)


def build_nc(T=2048, s_fp8=S_FP8, gates=None):
    NQ = T // QBLK
    NK = T // KBLK
    nc = bacc.Bacc(None)

    # x and w_attn arrive as hi/lo fp8 pairs: x ~= xth + xtl (lo holds the
    # quantization residual, subnormal-heavy), w_attn scaled by 32 so its
    # hi part uses the e4m3 normal range; the 1/32 is unapplied in the
    # q/k/v psum evacuations.  Projections run as three fp8 DoubleRow
    # chains (hi.hi + hi.lo + lo.hi) at 2x the fp16 matmul rate.
    xth_d = nc.dram_tensor("xth", [C, T], F8, kind="ExternalInput")
    xtl_d = nc.dram_tensor("xtl", [C, T], F8, kind="ExternalInput")
    # per-projection weights, host-PRE-SHUFFLED to [r=128, (c2 j m)] so
    # both DMA sides are 2304B-contiguous (full-rate; a 384B-row slice
    # transfer pays the <512B half-rate penalty)
    wqkv_d = {}
    for nm in ("wqh", "wql", "wkh", "wkl", "wvh", "wvl"):
        wqkv_d[nm] = nc.dram_tensor(nm, [128, 6 * GC], F8,
                                    kind="ExternalInput")
    wp_d = nc.dram_tensor("wp", [GC, C], F16, kind="ExternalInput")
    out_d = nc.dram_tensor("out", [T, C], F16, kind="ExternalOutput")
    W_UNSCALE = 1.0 / 32.0

    qk_dt = F8 if s_fp8 else F16
    exp_scale = 0.0625 if s_fp8 else 0.125

    with ExitStack() as ctx:
        tc = ctx.enter_context(tile.TileContext(nc))
        const = ctx.enter_context(tc.tile_pool(name="const", bufs=1))
        big = ctx.enter_context(tc.tile_pool(name="big", bufs=1))
        xtp = ctx.enter_context(tc.tile_pool(name="xtp", bufs=4))
        ptp = ctx.enter_context(tc.tile_pool(name="ptp", bufs=3))
        yqp = ctx.enter_context(tc.tile_pool(name="yqp", bufs=3))
        recp = ctx.enter_context(tc.tile_pool(name="recp", bufs=3))
        ytp = ctx.enter_context(tc.tile_pool(name="ytp", bufs=4))
        obp = ctx.enter_context(tc.tile_pool(name="obp", bufs=2))
        psS = ctx.enter_context(tc.tile_pool(name="psS", bufs=2, space="PSUM"))
        psY = ctx.enter_context(tc.tile_pool(name="psY", bufs=2, space="PSUM"))
        psQ = ctx.enter_context(tc.tile_pool(name="psQ", bufs=2, space="PSUM"))

        # constants.  ident first: the PE warmup matmuls depend on it, and
        # everything later on the pool queue (SWDGE descriptor generation,
        # mask builds) would delay it by ~3us.
        ident = const.tile([128, 128], F16)
        make_identity(nc, ident)
        mask2 = const.tile([128, 2, KBLK], F16)   # causal keep-mask, 2 heads
        make_upper_triangular(nc, mask2[:, 0, :], val=1.0, diag=True)
        make_upper_triangular(nc, mask2[:, 1, :], val=1.0, diag=True)

        # persistent.  weight/x layout for 256-contraction DoubleRow:
        # [r=128, ct2 in 3, j in 2, cols]; contraction c = ct2*256+j*128+r.
        CT2 = 3
        w8 = {}   # (proj, hl) -> [128, CT2, 2, GC] tile
        for proj in "qkv":
            for hl in (0, 1):
                w8[proj, hl] = big.tile([128, CT2, 2, GC], F8,
                                        name=f"w8{proj}{hl}")
        wp = big.tile([128, NP, C], F16)
        kt8 = big.tile([128, NP, T], qk_dt)
        qt8 = big.tile([128, NQ, NP, QBLK], qk_dt)
        # V with trailing ones column per (ktile, head): [k, 66] rows
        vs = big.tile([128, NK, HG, D + 2], F16)
        nc.gpsimd.memset(vs[:, :, :, D:D + 1], 1.0)

        xt_r = {0: xth_d[:, :].rearrange("(c2 j r) t -> r c2 j t",
                                         r=128, j=2),
                1: xtl_d[:, :].rearrange("(c2 j r) t -> r c2 j t",
                                         r=128, j=2)}
        # (xw)-chain list: (x hi/lo, w hi/lo)
        CHAINS = ((0, 0), (0, 1), (1, 0))
        wp_r = wp_d[:, :].rearrange("(p r) e -> r p e", r=128)
        out_r = out_d[:, :].rearrange("(q tt r) e -> q r tt e", tt=QBLK // KBLK,
                                      r=128)

        def dr_ap(ap):
            """[64, N] fp8 AP -> [64, 2, N] stride-0 DoubleRow operand."""
            return ap.unsqueeze(1).broadcast_to(
                [ap.shape[0], 2] + list(ap.shape[1:]))

        # ---- chunk helpers (each chunk = (callable, pe_ns), issued later) --
        PE_NS = 1.0 / 2.4   # ns per PE cycle at full speed
        work = deque()

        pe_ord = [0]   # PE event ordinal (Ldweights+Matmult pairs)

        def MM(*a, **k):
            pe_ord[0] += 2
            return nc.tensor.matmul(*a, **k)

        xtqs = {}

        def qk_chunks(q):
            """DMA + Q/K projection groups for tile q (critical early path)."""
            qs = q * QBLK

            def dma_x():
                xtq = xtp.tile([128, 2, CT2, 2, QBLK], F8, tag="xtq",
                               name="xtq")
                xtqs[q] = xtq
                for hl in (0, 1):   # hi first: half-0 chains need only hi
                    nc.sync.dma_start(out=xtq[:, hl, :, :, :],
                                      in_=xt_r[hl][:, :, :, qs:qs + QBLK])

            chunks = [(dma_x, 0.0, f"dx:{q}")]

            pss = {}

            def qk_part(p, which, half):
                if half == 0:
                    pss[(p, which)] = psQ.tile([128, QBLK], F32, tag="pq",
                                               name="pqk")
                ps = pss[(p, which)]
                proj = "q" if which == 0 else "k"
                # half 0: the hi.hi chain; half 1: the two cross chains
                chains = CHAINS[0:1] if half == 0 else CHAINS[1:3]
                for ci, (xs, ws) in enumerate(chains):
                    for ct2 in range(CT2):
                        first = half == 0 and ci == 0 and ct2 == 0
                        last = half == 1 and ci == len(chains) - 1 \
                            and ct2 == CT2 - 1
                        MM(ps, lhsT=w8[proj, ws][:, ct2, :,
                                                 p * 128:(p + 1) * 128],
                           rhs=xtqs[q][:, xs, ct2, :, :],
                           start=first, stop=last, perf_mode=DR)
                if half == 1:
                    del pss[(p, which)]
                    if which == 0:
                        nc.scalar.mul(qt8[:, q, p, :], ps, W_UNSCALE)
                    else:
                        nc.scalar.mul(kt8[:, p, qs:qs + QBLK], ps, W_UNSCALE)

            # half-0 (hi.hi) chains first within each pair: they need only
            # the hi tensors, which DMA-land first
            for p in range(NP):
                for half in (0, 1):
                    for which in (1, 0):   # K first
                        chunks.append(
                            (lambda p=p, w=which, h=half: qk_part(p, w, h),
                             (3 if half == 0 else 6) * 256 * PE_NS,
                             f"qk:{q}:{p}:{which}" if half else
                             f"qka:{q}:{p}:{which}"))
            return chunks

        def v_chunks(q):
            """V projection groups for tile q (needed by PV, not by S/exp)."""

            pss = {}

            def v_part(kl, half):
                k_i = 4 * q + kl
                if half == 0:
                    pss[kl] = psQ.tile([128, QBLK], F32, tag="pq", name="pv")
                ps = pss[kl]
                chains = CHAINS[0:1] if half == 0 else CHAINS[1:3]
                for ci, (xs, ws) in enumerate(chains):
                    for ct2 in range(CT2):
                        first = half == 0 and ci == 0 and ct2 == 0
                        last = half == 1 and ci == len(chains) - 1 \
                            and ct2 == CT2 - 1
                        MM(ps[:, 0:GC],
                           lhsT=xtqs[q][:, xs, ct2, :,
                                        kl * KBLK:(kl + 1) * KBLK],
                           rhs=w8["v", ws][:, ct2, :, :],
                           start=first, stop=last, perf_mode=DR)
                if half == 1:
                    del pss[kl]
                    nc.scalar.mul(
                        vs[:, k_i, :, 0:D],
                        ps[:, 0:GC].rearrange("r (h d) -> r h d", d=D),
                        W_UNSCALE)

            return [(lambda kl=kl, h=h: v_part(kl, h),
                     (3 if h == 0 else 6) * (GC // 2) * PE_NS,
                     f"v:{q}:{kl}" if h else f"va:{q}:{kl}")
                    for kl in range(4) for h in (0, 1)]

        # engine assignment for S-stream evacuation.  strict alternation:
        # consecutive evacuations on the same engine serialize (~1.1us
        # each) while alternating ones overlap, halving the S-stream
        # period.  ALT_PATTERN cycles act/dve; deficit mode weights by
        # SHARE instead.
        eng_credit = {"act": 0.0, "dve": 0.0}
        alt_state = [0]

        def pick_engine(elems, q=None):
            if ALTERNATE:
                e = ("act", "dve")[alt_state[0] % 2]
                alt_state[0] += 1
                return e
            a = SHARE_BY_Q.get(q, SHARE["act"])
            shares = {"act": a, "dve": 1.0 - a}
            for e in eng_credit:
                eng_credit[e] += shares[e] * elems
            best = max(eng_credit, key=lambda e: eng_credit[e])
            eng_credit[best] -= elems
            return best

        def s_evac(eng, dst, src):
            if eng == "act":
                nc.scalar.activation(dst, src, AF.Exp, scale=exp_scale)
            else:
                nc.vector.tensor_scalar(dst.bitcast(I16), src,
                                        A_SCHR, B_SCHR,
                                        op0=ALU.mult, op1=ALU.add)

        def s_batch(q, p, k_i, pt):
            """S^T (both heads) + exp + mask for one k-tile."""
            col0 = max(k_i - 4 * q, 0) * KBLK
            st = psS.tile([128, 2, QBLK], F32, tag="st", name="st")
            for s in range(2):
                hoff = D * s
                lhsT = kt8[hoff:hoff + D, p, k_i * KBLK:(k_i + 1) * KBLK]
                rhs = qt8[hoff:hoff + D, q, p, col0:QBLK]
                MM(st[:, s, col0:QBLK],
                   lhsT=dr_ap(lhsT), rhs=dr_ap(rhs),
                   start=True, stop=True, perf_mode=DR)
            if q == 3 and p == 2 and k_i >= 14:
                # program-final batches: act (the faster evacuator) so the
                # retire tail starts as early as possible
                eng = "act"
            else:
                eng = pick_engine(2 * (QBLK - col0), q)
            s_evac(eng, pt[:, k_i, :, col0:QBLK], st[:, :, col0:QBLK])
            if k_i >= 4 * q:   # diagonal tile: zero below-diagonal
                seg = pt[:, k_i, :, col0:col0 + KBLK]
                nc.gpsimd.tensor_mul(seg, seg, mask2)

        yts = {}
        tile_stages_done = {}

        def retire_chunks(q, p, pt, last=False):
            """PV + normalize chunks; transpose/c_proj go to `late` (they
            depend on DVE results of the PV chunks — spacing them a stage
            later avoids PE head-of-line stalls).  For the final stage
            (`last`), everything chains per token-subtile instead so the
            post-last-exp critical path covers one subtile, not four."""
            chunks = []
            late = []
            yas = {}
            yq4s = []

            def pv_group(s, tt):
                if tt == 0:
                    yas[s] = psY.tile([128, 4, KBLK], F32, tag="y", name="ya")
                ya = yas[s]
                h = p * 2 + s
                nkt = 4 * q + tt + 1
                for k_i in range(nkt):
                    MM(
                        ya[:, tt, 0:D + 1],
                        lhsT=pt[:, k_i, s, tt * KBLK:(tt + 1) * KBLK],
                        rhs=vs[:, k_i, h, 0:D + 1],
                        start=(k_i == 0), stop=(k_i == nkt - 1),
                        skip_group_check=True)

            def norm(s):
                # y * (1/rowsum), per token-subtile
                if not yq4s:
                    yq4s.append(yqp.tile([128, 4, 128], F16, tag="yq",
                                         name="yq"))
                ya = yas.pop(s)
                rec = recp.tile([128, 4], F32, tag="rec", name="rec")
                nc.vector.reciprocal_approx_fast(rec, ya[:, :, D:D + 1])
                for tt in range(4):
                    nc.vector.tensor_scalar(
                        yq4s[0][:, tt, s * D:(s + 1) * D], ya[:, tt, 0:D],
                        rec[:, tt:tt + 1], None, op0=ALU.mult)

            def transpose_all():
                # y^T via the DMA XBAR (14ns per 32x32 tile, runs on the
                # mostly-idle DMA engines; frees PE + DVE + a psum bank)
                if q not in yts:
                    yts[q] = ytp.tile([128, NP, QBLK], F16, tag="yt",
                                      name="yt")
                for tt in range(4):
                    nc.sync.dma_start_transpose(
                        yts[q][:, p, tt * KBLK:(tt + 1) * KBLK],
                        yq4s[0][:, tt, :])

            def norm_tt(tt):
                if not yq4s:
                    yq4s.append(yqp.tile([128, 4, 128], F16, tag="yq",
                                         name="yq"))
                rec = recp.tile([128, 2], F32, tag="rec", name="rec")
                for s in range(2):
                    nc.vector.reciprocal_approx_fast(
                        rec[:, s:s + 1], yas[s][:, tt, D:D + 1])
                    nc.vector.tensor_scalar(
                        yq4s[0][:, tt, s * D:(s + 1) * D],
                        yas[s][:, tt, 0:D],
                        rec[:, s:s + 1], None, op0=ALU.mult)

            def transpose_tt(tt):
                if q not in yts:
                    yts[q] = ytp.tile([128, NP, QBLK], F16, tag="yt",
                                      name="yt")
                tp = psQ.tile([128, KBLK], F16, tag="pq", name="tp")
                pe_ord[0] += 2
                nc.tensor.transpose(tp, yq4s[0][:, tt, :], ident)
                nc.vector.tensor_copy(
                    yts[q][:, p, tt * KBLK:(tt + 1) * KBLK], tp)

            if not last:
                for s in range(2):
                    for tt in range(4):
                        chunks.append((lambda s=s, tt=tt: pv_group(s, tt),
                                       (4 * q + tt + 1) * (D + 1) * PE_NS,
                                       f"pv:{q}:{p}:{s}:{tt}"))
                    chunks.append((lambda s=s: norm(s), 0.0,
                                   f"nm:{q}:{p}:{s}"))
                tps = [(transpose_all, 0.0, f"tp:{q}:{p}")]
            else:
                for tt in range(4):
                    for s in range(2):
                        chunks.append((lambda s=s, tt=tt: pv_group(s, tt),
                                       (4 * q + tt + 1) * (D + 1) * PE_NS,
                                       f"pv:{q}:{p}:{s}:{tt}"))
                    chunks.append((lambda tt=tt: norm_tt(tt), 0.0,
                                   f"nm:{q}:{p}:{tt // 3}"))
                    chunks.append((lambda tt=tt: transpose_tt(tt),
                                   128 * PE_NS + 70.0, f"tp:{q}:{p}:{tt}"))
                tps = []

            tile_stages_done[q] = tile_stages_done.get(q, 0) + 1
            late.extend(tps)
            if tile_stages_done[q] == NP:
                obs_local = {}

                def cproj2(tt, ec):
                    if q not in obs_local:
                        obs_local[q] = obp.tile([128, 4, C], F16, tag="ob",
                                                name="ob")
                    po = psQ.tile([128, QBLK], F32, tag="pq", name="po")
                    yt = yts[q]
                    for j in range(NP):
                        MM(
                            po[:, 0:GC],
                            lhsT=yt[:, j, tt * KBLK:(tt + 1) * KBLK],
                            rhs=wp[:, j, ec * GC:(ec + 1) * GC],
                            start=(j == 0), stop=(j == NP - 1))
                    dst = obs_local[q][:, tt, ec * GC:(ec + 1) * GC]
                    if q >= 2:
                        # late tiles: act is idle in the retire tail while
                        # dve is the bottleneck there
                        nc.scalar.copy(dst, po[:, 0:GC])
                    else:
                        nc.vector.tensor_copy(dst, po[:, 0:GC])

                def out_dma(tt):
                    nc.sync.dma_start(out=out_r[q][:, tt, :],
                                      in_=obs_local[q][:, tt, :])
                    if tt == 3:
                        yts.pop(q)
                        obs_local.pop(q)

                def out_dma_ec(ec):
                    # program-final subtile: per-ec DMA right after its
                    # evac shortens the very last evac->dma->sem chain
                    nc.sync.dma_start(
                        out=out_r[q][:, 3, ec * GC:(ec + 1) * GC],
                        in_=obs_local[q][:, 3, ec * GC:(ec + 1) * GC])
                    if ec == 1:
                        yts.pop(q)
                        obs_local.pop(q)

                cpod = [[] for _ in range(4)]
                for tt in range(4):
                    for ec in range(2):
                        cpod[tt].append(
                            (lambda tt=tt, ec=ec: cproj2(tt, ec),
                             NP * GC * PE_NS, f"cp:{q}:{tt}:{ec}"))
                        if last and tt == 3:
                            cpod[tt].append(
                                (lambda ec=ec: out_dma_ec(ec), 0.0,
                                 f"od:{q}:3:{ec}"))
                    if not (last and tt == 3):
                        cpod[tt].append((lambda tt=tt: out_dma(tt), 0.0,
                                         f"od:{q}:{tt}"))
                if last:
                    # two-step skew: pv(tt) || norm+tp(tt-1) || c_proj(tt-2)
                    # so PE never waits a full DVE chain between subtiles
                    grp = [chunks[4 * tt:4 * tt + 4] for tt in range(4)]
                    newc = []
                    for step in range(6):
                        if step < 4:
                            newc.extend(grp[step][0:2])      # pv pair
                        if 1 <= step <= 4:
                            newc.extend(grp[step - 1][2:4])  # norm, tp
                        if step >= 2:
                            newc.extend(cpod[step - 2])
                    chunks[:] = newc
                else:
                    for tt in range(4):
                        late.extend(cpod[tt])
            return chunks, late

        # ---- main pipelined issue loop ----
        issued = set()

        # Queue A: Q/K projections of ALL tiles (critical path: enables
        # Act's late-tile exp work early).  Queue B: V projections and
        # retire work — drained in the Act-bound phase where PE has slack.
        workA = work
        workB = deque()

        def pop_work():
            src = workA if workA else workB
            chunk, cost, label = src.popleft()
            chunk()
            issued.add(label)
            return cost, label

        def pop_workB():
            chunk, cost, label = workB.popleft()
            chunk()
            issued.add(label)
            return cost, label

        # weight DMAs.  HWDGE is a single shared descriptor generator, so
        # issue order across queues IS landing order: K-hi first (the first
        # qk chain needs it), then x (tile 0, via qk_chunks' dma_x on sync),
        # then Q-hi and the lo parts on the Act queue (whose seq is busy
        # with LoadActFuncSet for the first ~2us anyway).  V and wp go on
        # the gpsimd SWDGE queue — pool is idle this early.
        # weight DMAs: K-hi first (first qk chain), x tile 0 next, then the
        # remaining tensors on the Act queue in need order.  All transfers
        # are [128, 2304]-contiguous on both sides (full DMA rate).
        nc.sync.dma_start(out=w8["k", 0], in_=wqkv_d["wkh"][:, :])
        workA.extend(qk_chunks(0))
        pop_work()   # x^T DMA of tile 0 — next on the sync queue
        nc.scalar.dma_start(out=w8["q", 0], in_=wqkv_d["wqh"][:, :])
        nc.scalar.dma_start(out=w8["k", 1], in_=wqkv_d["wkl"][:, :])
        nc.scalar.dma_start(out=w8["q", 1], in_=wqkv_d["wql"][:, :])
        nc.scalar.dma_start(out=w8["v", 0], in_=wqkv_d["wvh"][:, :])
        nc.scalar.dma_start(out=w8["v", 1], in_=wqkv_d["wvl"][:, :])
        nc.gpsimd.dma_start(out=wp, in_=wp_r)
        # warm the PE p-state while the first DMAs are in flight: dummy
        # matmuls on a const tile keep the array continuously busy so the
        # real Q/K projections start at full clock
        junk = const.tile([128, QBLK], F16)
        nc.vector.memset(junk, 0.0)
        for _ in range(WARMUP):
            jp = psS.tile([128, 2, QBLK], F32, tag="st", name="jp")
            MM(jp[:, 0, :], lhsT=ident, rhs=junk, start=True, stop=True)
        for _ in range(4):   # Q/K of pair 0 eagerly
            pop_work()
        for q in range(1, NQ):
            workA.extend(qk_chunks(q))

        # drain budgets per global S-batch index (measured-stall feedback);
        # records of what was actually drained are kept for the tuner.
        drained_rec = []
        marks = []   # PE event ordinal at the start of each S batch

        stages = [(q, p) for q in range(NQ) for p in range(NP)]
        if HOLDOUT and NQ > 1:
            # hold one small early stage for the end: its S/exp stream hides
            # the last big tile's c_proj, and its own retire tail is short
            stages.remove(HOLDOUT)
            stages.append(HOLDOUT)
        pend_late = []
        b = 0   # global S-batch index
        for i, (q, p) in enumerate(stages):
            if p == 0:
                workB.extend(v_chunks(q))
            # PE-order safety: this stage's Q/K groups must be issued first
            while (f"qk:{q}:{p}:0" not in issued
                   or f"qk:{q}:{p}:1" not in issued):
                pop_work()
            # pt-pool WAR safety: this stage's exp writes reuse the pt slot
            # of stage i-3 — its PV/norm chunks must already be issued, or
            # Act would wait on PE work scheduled after this stage
            if i >= 3:
                oq, op = stages[i - 3]
                while f"nm:{oq}:{op}:1" not in issued:
                    pop_workB()
            nk = 4 * (q + 1)
            pt = ptp.tile([128, nk, 2, QBLK], F16, tag="pt", name="pt")
            for k_i in range(nk):
                budget = gates[b] if gates is not None and b < len(gates) \
                    else DRAIN_NS
                spent = 0.0
                while workA or workB:
                    nxt = (workA or workB)[0][1]
                    if spent + max(nxt, 60.0) > budget + 200.0:
                        break
                    c, lab = pop_work()
                    spent += max(c, 60.0)
                drained_rec.append(spent)
                marks.append(pe_ord[0])
                s_batch(q, p, k_i, pt)
                b += 1
            if i == len(stages) - 1:
                # final stage: prior pair's transposes must precede its
                # per-subtile c_proj chains in issue order
                chunks, late = retire_chunks(q, p, pt, last=True)
                workB.extend(pend_late)
                workB.extend(chunks)
            else:
                chunks, late = retire_chunks(q, p, pt)
                workB.extend(chunks)
                workB.extend(pend_late)
            pend_late = late
        workB.extend(pend_late)
        while workA or workB:
            pop_work()

        build_nc.last_drained = drained_rec
        build_nc.last_marks = marks

    nc.compile()
    return nc


def make_in_map(x_b, w_attn, w_proj, g):
    """Per-core input arrays for batch slice x_b and head-group g."""
    import ml_dtypes
    E4 = ml_dtypes.float8_e4m3fn

    def shuf(w):
        # [768, 384] -> [r=128, (c2 j m)=2304]: row r holds the weights
        # for contraction rows c2*256 + j*128 + r, matching the on-device
        # DoubleRow tile layout with a fully contiguous DMA
        return np.ascontiguousarray(
            w.reshape(3, 2, 128, GC).transpose(2, 0, 1, 3).reshape(128, -1))

    sl = slice(g * GC, (g + 1) * GC)
    out = {}
    for nm, w in (("wq", w_attn[:, 0 * C:1 * C][:, sl]),
                  ("wk", w_attn[:, 1 * C:2 * C][:, sl]),
                  ("wv", w_attn[:, 2 * C:3 * C][:, sl])):
        ww = np.ascontiguousarray(w).astype(np.float32) * 32.0
        hi = ww.astype(E4)
        lo = (ww - hi.astype(np.float32)).astype(E4)
        out[f"w{nm[1]}h"] = shuf(hi)
        out[f"w{nm[1]}l"] = shuf(lo)
    xt = np.ascontiguousarray(x_b.T).astype(np.float32)
    xth = xt.astype(E4)
    out["xth"] = xth
    out["xtl"] = (xt - xth.astype(np.float32)).astype(E4)
    out["wp"] = np.ascontiguousarray(w_proj[sl, :]).astype(np.float16)
    return out


_NC_CACHE = {}


def _get_nc(T):
    if T not in _NC_CACHE:
        _NC_CACHE[T] = build_nc(T)
    return _NC_CACHE[T]


def kernel(x, w_attn, b_attn, w_proj, b_proj, _trace=False):
    from concourse.bass_utils import run_bass_kernel_spmd

    x = np.asarray(x, dtype=np.float32)
    w_attn = np.asarray(w_attn, dtype=np.float32)
    b_attn = np.asarray(b_attn, dtype=np.float32)
    w_proj = np.asarray(w_proj, dtype=np.float32)
    b_proj = np.asarray(b_proj, dtype=np.float32)
    B, T, _ = x.shape

    assert not np.any(b_attn[0:2 * C] != 0.0), \
        "nonzero q/k bias not supported by this kernel"

    nc = _get_nc(T)
    in_maps = []
    for b in range(B):
        for g in range(2):
            in_maps.append(make_in_map(x[b], w_attn, w_proj, g))
    res = run_bass_kernel_spmd(nc, in_maps, core_ids=list(range(2 * B)),
                               trace=_trace)
    outs = [np.asarray(r["out"], dtype=np.float32) for r in res.results]
    # softmax rows sum to 1, so the V-bias contribution is exactly
    # bv @ w_proj added to every token (not computed on device).
    bias_row = b_proj + b_attn[2 * C:3 * C] @ w_proj
    out = np.empty((B, T, C), dtype=np.float32)
    for b in range(B):
        out[b] = outs[2 * b] + outs[2 * b + 1] + bias_row[None, :]
    if _trace:
        kernel.last_result = res
    return out



# revision 75
# speedup vs baseline: 1.0026x; 1.0003x over previous
"""Causal self-attention (B=4 T=2048 C=768 H=12) on 8 trn2 cores — v3.

Sharding: core = (batch b, head-group g), g in {0,1} covering 6 heads
(3 pairs).  Host sums the two partial c_proj outputs per batch and adds
the exact bias row (b_proj + bv @ w_proj; softmax rows sum to 1).

v3 design (vs v2, 140850 -> 130486 ns):
  * hi/lo fp8 QKV projections: host splits x and 32*w_attn into e4m3
    hi + residual-lo (subnormal) pairs; three 256-contraction DoubleRow
    chains (hi.hi + hi.lo + lo.hi) in one psum accumulation run at 2x
    the fp16 rate with ~0.14% error.  1/32 unapplied in the evacuations.
  * S^T = K^T.Q per (pair, head) via fp8e4m3 DoubleRow matmuls at 0.5
    cycles/row (contraction-64 as 2 broadcast slots; exp scale 0.0625
    absorbs the doubling).
  * S evacuation split across engines (pool is barred from PSUM): act
    does true exp for ~58% of elements, dve does a 1-op fp16-bitcast
    schraudolph exp (tensor_scalar mult/add into an int16-bitcast view,
    1.8% RMS on its share; tile-scattered so end-to-end cost is ~5e-4).
    Per-q act shares lean higher for late tiles (dve carries retire).
  * causal masks on the pool engine (SBUF-only); y normalization via
    reciprocal + per-subtile tensor_scalar on dve; y^T via XBAR
    transpose-DMAs on the sync queue (PE/DVE freed); c_proj evacuation
    on dve early / act late (act idles in the tail).
  * chunk-queue software pipelining with a small (160ns) per-S-batch
    drain budget; weight/x DMA issue order tuned against the single
    serializing DMA_ENGINES device (K-hi, x0, Q-hi, los, V, wp).
"""

from collections import deque
from contextlib import ExitStack

import numpy as np

import concourse.bass as bass
import concourse.mybir as mybir
import concourse.tile as tile
from concourse import bacc
from concourse.masks import make_upper_triangular, make_identity

AF = mybir.ActivationFunctionType
ALU = mybir.AluOpType
F32 = mybir.dt.float32
F16 = mybir.dt.float16
F8 = mybir.dt.float8e4
I16 = mybir.dt.int16
DR = mybir.MatmulPerfMode.DoubleRow

# fp16-bitcast schraudolph exp approximation: exp(s*scale) ~=
# bitcast16(int16(A_SCHR*s + B_SCHR)).  C=-0.0575 minimizes RMS rel err
# (1.78%); +0.5 turns the trunc-on-int-store into round-to-nearest.
SCHR_SCALE = 0.0625          # fp8-DR doubled psum units
A_SCHR = float(np.float32(1024.0 * 1.4426950408889634 * SCHR_SCALE))
B_SCHR = float(np.float32(15 * 1024 + 1024.0 * (-0.0575) + 0.5))

# engine shares for the S-stream evacuation (fraction of elements).
# pool cannot touch PSUM (hard BIR rule), so the split is act (true exp,
# 0.833/elem) vs dve (schraudolph, 1.04/elem on fp32 psum).
SHARE = {"act": 0.60, "dve": 0.40}
# per-q act-share overrides: late (big) stages run while dve also carries
# the retire stream, so they lean harder on act
SHARE_BY_Q = {0: 0.54, 1: 0.59, 2: 0.61, 3: 0.63}
ALTERNATE = False   # strict act/dve alternation for S evacuation

C = 768          # model dim
D = 64           # head dim
HG = 6           # heads per core
NP = 3           # head pairs per core
GC = HG * D      # 384 group channels
CT = C // 128    # 6 contraction tiles
QBLK = 512       # query tile (psum bank)
KBLK = 128       # key tile

S_FP8 = True     # fp8e4m3 DoubleRow for the S matmul (else fp16)
WARMUP = 12      # PE p-state warmup matmuls at startup
HOLDOUT = None   # stage held to the end to shorten the tail (None = off)
DRAIN_NS = 160.0  # default per-S-batch PE-work drain budget (gates override)


def build_nc(T=2048, s_fp8=S_FP8, gates=None):
    NQ = T // QBLK
    NK = T // KBLK
    nc = bacc.Bacc(None)

    # x and w_attn arrive as hi/lo fp8 pairs: x ~= xth + xtl (lo holds the
    # quantization residual, subnormal-heavy), w_attn scaled by 32 so its
    # hi part uses the e4m3 normal range; the 1/32 is unapplied in the
    # q/k/v psum evacuations.  Projections run as three fp8 DoubleRow
    # chains (hi.hi + hi.lo + lo.hi) at 2x the fp16 matmul rate.
    xth_d = nc.dram_tensor("xth", [C, T], F8, kind="ExternalInput")
    xtl_d = nc.dram_tensor("xtl", [C, T], F8, kind="ExternalInput")
    # per-projection weights, host-PRE-SHUFFLED to [r=128, (c2 j m)] so
    # both DMA sides are 2304B-contiguous (full-rate; a 384B-row slice
    # transfer pays the <512B half-rate penalty)
    wqkv_d = {}
    for nm in ("wqh", "wql", "wkh", "wkl", "wvh", "wvl"):
        wqkv_d[nm] = nc.dram_tensor(nm, [128, 6 * GC], F8,
                                    kind="ExternalInput")
    wp_d = nc.dram_tensor("wp", [GC, C], F16, kind="ExternalInput")
    out_d = nc.dram_tensor("out", [T, C], F16, kind="ExternalOutput")
    W_UNSCALE = 1.0 / 32.0

    qk_dt = F8 if s_fp8 else F16
    exp_scale = 0.0625 if s_fp8 else 0.125

    with ExitStack() as ctx:
        tc = ctx.enter_context(tile.TileContext(nc))
        const = ctx.enter_context(tc.tile_pool(name="const", bufs=1))
        big = ctx.enter_context(tc.tile_pool(name="big", bufs=1))
        xtp = ctx.enter_context(tc.tile_pool(name="xtp", bufs=4))
        ptp = ctx.enter_context(tc.tile_pool(name="ptp", bufs=3))
        yqp = ctx.enter_context(tc.tile_pool(name="yqp", bufs=3))
        recp = ctx.enter_context(tc.tile_pool(name="recp", bufs=3))
        ytp = ctx.enter_context(tc.tile_pool(name="ytp", bufs=4))
        obp = ctx.enter_context(tc.tile_pool(name="obp", bufs=2))
        psS = ctx.enter_context(tc.tile_pool(name="psS", bufs=2, space="PSUM"))
        psY = ctx.enter_context(tc.tile_pool(name="psY", bufs=2, space="PSUM"))
        psQ = ctx.enter_context(tc.tile_pool(name="psQ", bufs=2, space="PSUM"))

        # constants.  ident first: the PE warmup matmuls depend on it, and
        # everything later on the pool queue (SWDGE descriptor generation,
        # mask builds) would delay it by ~3us.
        ident = const.tile([128, 128], F16)
        make_identity(nc, ident)
        mask2 = const.tile([128, 2, KBLK], F16)   # causal keep-mask, 2 heads
        make_upper_triangular(nc, mask2[:, 0, :], val=1.0, diag=True)
        make_upper_triangular(nc, mask2[:, 1, :], val=1.0, diag=True)

        # persistent.  weight/x layout for 256-contraction DoubleRow:
        # [r=128, ct2 in 3, j in 2, cols]; contraction c = ct2*256+j*128+r.
        CT2 = 3
        w8 = {}   # (proj, hl) -> [128, CT2, 2, GC] tile
        for proj in "qkv":
            for hl in (0, 1):
                w8[proj, hl] = big.tile([128, CT2, 2, GC], F8,
                                        name=f"w8{proj}{hl}")
        wp = big.tile([128, NP, C], F16)
        kt8 = big.tile([128, NP, T], qk_dt)
        qt8 = big.tile([128, NQ, NP, QBLK], qk_dt)
        # V with trailing ones column per (ktile, head): [k, 66] rows
        vs = big.tile([128, NK, HG, D + 2], F16)
        nc.gpsimd.memset(vs[:, :, :, D:D + 1], 1.0)

        xt_r = {0: xth_d[:, :].rearrange("(c2 j r) t -> r c2 j t",
                                         r=128, j=2),
                1: xtl_d[:, :].rearrange("(c2 j r) t -> r c2 j t",
                                         r=128, j=2)}
        # (xw)-chain list: (x hi/lo, w hi/lo)
        CHAINS = ((0, 0), (0, 1), (1, 0))
        wp_r = wp_d[:, :].rearrange("(p r) e -> r p e", r=128)
        out_r = out_d[:, :].rearrange("(q tt r) e -> q r tt e", tt=QBLK // KBLK,
                                      r=128)

        def dr_ap(ap):
            """[64, N] fp8 AP -> [64, 2, N] stride-0 DoubleRow operand."""
            return ap.unsqueeze(1).broadcast_to(
                [ap.shape[0], 2] + list(ap.shape[1:]))

        # ---- chunk helpers (each chunk = (callable, pe_ns), issued later) --
        PE_NS = 1.0 / 2.4   # ns per PE cycle at full speed
        work = deque()

        pe_ord = [0]   # PE event ordinal (Ldweights+Matmult pairs)

        def MM(*a, **k):
            pe_ord[0] += 2
            return nc.tensor.matmul(*a, **k)

        xtqs = {}

        def qk_chunks(q):
            """DMA + Q/K projection groups for tile q (critical early path)."""
            qs = q * QBLK

            def dma_x():
                xtq = xtp.tile([128, 2, CT2, 2, QBLK], F8, tag="xtq",
                               name="xtq")
                xtqs[q] = xtq
                for hl in (0, 1):   # hi first: half-0 chains need only hi
                    nc.sync.dma_start(out=xtq[:, hl, :, :, :],
                                      in_=xt_r[hl][:, :, :, qs:qs + QBLK])

            chunks = [(dma_x, 0.0, f"dx:{q}")]

            pss = {}

            def qk_part(p, which, half):
                if half == 0:
                    pss[(p, which)] = psQ.tile([128, QBLK], F32, tag="pq",
                                               name="pqk")
                ps = pss[(p, which)]
                proj = "q" if which == 0 else "k"
                # half 0: the hi.hi chain; half 1: the two cross chains
                chains = CHAINS[0:1] if half == 0 else CHAINS[1:3]
                for ci, (xs, ws) in enumerate(chains):
                    for ct2 in range(CT2):
                        first = half == 0 and ci == 0 and ct2 == 0
                        last = half == 1 and ci == len(chains) - 1 \
                            and ct2 == CT2 - 1
                        MM(ps, lhsT=w8[proj, ws][:, ct2, :,
                                                 p * 128:(p + 1) * 128],
                           rhs=xtqs[q][:, xs, ct2, :, :],
                           start=first, stop=last, perf_mode=DR)
                if half == 1:
                    del pss[(p, which)]
                    if which == 0:
                        nc.scalar.mul(qt8[:, q, p, :], ps, W_UNSCALE)
                    else:
                        nc.scalar.mul(kt8[:, p, qs:qs + QBLK], ps, W_UNSCALE)

            # half-0 (hi.hi) chains first within each pair: they need only
            # the hi tensors, which DMA-land first
            for p in range(NP):
                for half in (0, 1):
                    for which in (1, 0):   # K first
                        chunks.append(
                            (lambda p=p, w=which, h=half: qk_part(p, w, h),
                             (3 if half == 0 else 6) * 256 * PE_NS,
                             f"qk:{q}:{p}:{which}" if half else
                             f"qka:{q}:{p}:{which}"))
            return chunks

        def v_chunks(q):
            """V projection groups for tile q (needed by PV, not by S/exp)."""

            pss = {}

            def v_part(kl, half):
                k_i = 4 * q + kl
                if half == 0:
                    pss[kl] = psQ.tile([128, QBLK], F32, tag="pq", name="pv")
                ps = pss[kl]
                chains = CHAINS[0:1] if half == 0 else CHAINS[1:3]
                for ci, (xs, ws) in enumerate(chains):
                    for ct2 in range(CT2):
                        first = half == 0 and ci == 0 and ct2 == 0
                        last = half == 1 and ci == len(chains) - 1 \
                            and ct2 == CT2 - 1
                        MM(ps[:, 0:GC],
                           lhsT=xtqs[q][:, xs, ct2, :,
                                        kl * KBLK:(kl + 1) * KBLK],
                           rhs=w8["v", ws][:, ct2, :, :],
                           start=first, stop=last, perf_mode=DR)
                if half == 1:
                    del pss[kl]
                    nc.scalar.mul(
                        vs[:, k_i, :, 0:D],
                        ps[:, 0:GC].rearrange("r (h d) -> r h d", d=D),
                        W_UNSCALE)

            return [(lambda kl=kl, h=h: v_part(kl, h),
                     (3 if h == 0 else 6) * (GC // 2) * PE_NS,
                     f"v:{q}:{kl}" if h else f"va:{q}:{kl}")
                    for kl in range(4) for h in (0, 1)]

        # engine assignment for S-stream evacuation.  strict alternation:
        # consecutive evacuations on the same engine serialize (~1.1us
        # each) while alternating ones overlap, halving the S-stream
        # period.  ALT_PATTERN cycles act/dve; deficit mode weights by
        # SHARE instead.
        eng_credit = {"act": 0.0, "dve": 0.0}
        alt_state = [0]

        def pick_engine(elems, q=None):
            if ALTERNATE:
                e = ("act", "dve")[alt_state[0] % 2]
                alt_state[0] += 1
                return e
            a = SHARE_BY_Q.get(q, SHARE["act"])
            shares = {"act": a, "dve": 1.0 - a}
            for e in eng_credit:
                eng_credit[e] += shares[e] * elems
            best = max(eng_credit, key=lambda e: eng_credit[e])
            eng_credit[best] -= elems
            return best

        def s_evac(eng, dst, src):
            if eng == "act":
                nc.scalar.activation(dst, src, AF.Exp, scale=exp_scale)
            else:
                nc.vector.tensor_scalar(dst.bitcast(I16), src,
                                        A_SCHR, B_SCHR,
                                        op0=ALU.mult, op1=ALU.add)

        def s_batch(q, p, k_i, pt):
            """S^T (both heads) + exp + mask for one k-tile."""
            col0 = max(k_i - 4 * q, 0) * KBLK
            st = psS.tile([128, 2, QBLK], F32, tag="st", name="st")
            for s in range(2):
                hoff = D * s
                lhsT = kt8[hoff:hoff + D, p, k_i * KBLK:(k_i + 1) * KBLK]
                rhs = qt8[hoff:hoff + D, q, p, col0:QBLK]
                MM(st[:, s, col0:QBLK],
                   lhsT=dr_ap(lhsT), rhs=dr_ap(rhs),
                   start=True, stop=True, perf_mode=DR)
            if q == 3 and p == 2 and k_i >= 14:
                # program-final batches: act (the faster evacuator) so the
                # retire tail starts as early as possible
                eng = "act"
            else:
                eng = pick_engine(2 * (QBLK - col0), q)
            s_evac(eng, pt[:, k_i, :, col0:QBLK], st[:, :, col0:QBLK])
            if k_i >= 4 * q:   # diagonal tile: zero below-diagonal
                seg = pt[:, k_i, :, col0:col0 + KBLK]
                nc.gpsimd.tensor_mul(seg, seg, mask2)

        yts = {}
        tile_stages_done = {}

        def retire_chunks(q, p, pt, last=False):
            """PV + normalize chunks; transpose/c_proj go to `late` (they
            depend on DVE results of the PV chunks — spacing them a stage
            later avoids PE head-of-line stalls).  For the final stage
            (`last`), everything chains per token-subtile instead so the
            post-last-exp critical path covers one subtile, not four."""
            chunks = []
            late = []
            yas = {}
            yq4s = []

            def pv_group(s, tt):
                if tt == 0:
                    yas[s] = psY.tile([128, 4, KBLK], F32, tag="y", name="ya")
                ya = yas[s]
                h = p * 2 + s
                nkt = 4 * q + tt + 1
                for k_i in range(nkt):
                    MM(
                        ya[:, tt, 0:D + 1],
                        lhsT=pt[:, k_i, s, tt * KBLK:(tt + 1) * KBLK],
                        rhs=vs[:, k_i, h, 0:D + 1],
                        start=(k_i == 0), stop=(k_i == nkt - 1),
                        skip_group_check=True)

            def norm(s):
                # y * (1/rowsum), per token-subtile
                if not yq4s:
                    yq4s.append(yqp.tile([128, 4, 128], F16, tag="yq",
                                         name="yq"))
                ya = yas.pop(s)
                rec = recp.tile([128, 4], F32, tag="rec", name="rec")
                nc.vector.reciprocal_approx_fast(rec, ya[:, :, D:D + 1])
                for tt in range(4):
                    nc.vector.tensor_scalar(
                        yq4s[0][:, tt, s * D:(s + 1) * D], ya[:, tt, 0:D],
                        rec[:, tt:tt + 1], None, op0=ALU.mult)

            def transpose_all():
                # y^T via the DMA XBAR (14ns per 32x32 tile, runs on the
                # mostly-idle DMA engines; frees PE + DVE + a psum bank)
                if q not in yts:
                    yts[q] = ytp.tile([128, NP, QBLK], F16, tag="yt",
                                      name="yt")
                for tt in range(4):
                    nc.sync.dma_start_transpose(
                        yts[q][:, p, tt * KBLK:(tt + 1) * KBLK],
                        yq4s[0][:, tt, :])

            def norm_tt(tt):
                if not yq4s:
                    yq4s.append(yqp.tile([128, 4, 128], F16, tag="yq",
                                         name="yq"))
                rec = recp.tile([128, 2], F32, tag="rec", name="rec")
                for s in range(2):
                    nc.vector.reciprocal_approx_fast(
                        rec[:, s:s + 1], yas[s][:, tt, D:D + 1])
                    nc.vector.tensor_scalar(
                        yq4s[0][:, tt, s * D:(s + 1) * D],
                        yas[s][:, tt, 0:D],
                        rec[:, s:s + 1], None, op0=ALU.mult)

            def transpose_tt(tt):
                if q not in yts:
                    yts[q] = ytp.tile([128, NP, QBLK], F16, tag="yt",
                                      name="yt")
                tp = psQ.tile([128, KBLK], F16, tag="pq", name="tp")
                pe_ord[0] += 2
                nc.tensor.transpose(tp, yq4s[0][:, tt, :], ident)
                nc.vector.tensor_copy(
                    yts[q][:, p, tt * KBLK:(tt + 1) * KBLK], tp)

            if not last:
                for s in range(2):
                    for tt in range(4):
                        chunks.append((lambda s=s, tt=tt: pv_group(s, tt),
                                       (4 * q + tt + 1) * (D + 1) * PE_NS,
                                       f"pv:{q}:{p}:{s}:{tt}"))
                    chunks.append((lambda s=s: norm(s), 0.0,
                                   f"nm:{q}:{p}:{s}"))
                tps = [(transpose_all, 0.0, f"tp:{q}:{p}")]
            else:
                for tt in range(4):
                    for s in range(2):
                        chunks.append((lambda s=s, tt=tt: pv_group(s, tt),
                                       (4 * q + tt + 1) * (D + 1) * PE_NS,
                                       f"pv:{q}:{p}:{s}:{tt}"))
                    chunks.append((lambda tt=tt: norm_tt(tt), 0.0,
                                   f"nm:{q}:{p}:{tt // 3}"))
                    chunks.append((lambda tt=tt: transpose_tt(tt),
                                   128 * PE_NS + 70.0, f"tp:{q}:{p}:{tt}"))
                tps = []

            tile_stages_done[q] = tile_stages_done.get(q, 0) + 1
            late.extend(tps)
            if tile_stages_done[q] == NP:
                obs_local = {}

                def cproj2(tt, ec):
                    if q not in obs_local:
                        obs_local[q] = obp.tile([128, 4, C], F16, tag="ob",
                                                name="ob")
                    po = psQ.tile([128, QBLK], F32, tag="pq", name="po")
                    yt = yts[q]
                    for j in range(NP):
                        MM(
                            po[:, 0:GC],
                            lhsT=yt[:, j, tt * KBLK:(tt + 1) * KBLK],
                            rhs=wp[:, j, ec * GC:(ec + 1) * GC],
                            start=(j == 0), stop=(j == NP - 1))
                    dst = obs_local[q][:, tt, ec * GC:(ec + 1) * GC]
                    if q >= 2:
                        # late tiles: act is idle in the retire tail while
                        # dve is the bottleneck there
                        nc.scalar.copy(dst, po[:, 0:GC])
                    else:
                        nc.vector.tensor_copy(dst, po[:, 0:GC])

                def out_dma(tt):
                    nc.sync.dma_start(out=out_r[q][:, tt, :],
                                      in_=obs_local[q][:, tt, :])
                    if tt == 3:
                        yts.pop(q)
                        obs_local.pop(q)

                def out_dma_ec(ec):
                    # program-final subtile: per-ec DMA right after its
                    # evac shortens the very last evac->dma->sem chain
                    nc.sync.dma_start(
                        out=out_r[q][:, 3, ec * GC:(ec + 1) * GC],
                        in_=obs_local[q][:, 3, ec * GC:(ec + 1) * GC])
                    if ec == 1:
                        yts.pop(q)
                        obs_local.pop(q)

                cpod = [[] for _ in range(4)]
                for tt in range(4):
                    for ec in range(2):
                        cpod[tt].append(
                            (lambda tt=tt, ec=ec: cproj2(tt, ec),
                             NP * GC * PE_NS, f"cp:{q}:{tt}:{ec}"))
                        if last and tt == 3:
                            cpod[tt].append(
                                (lambda ec=ec: out_dma_ec(ec), 0.0,
                                 f"od:{q}:3:{ec}"))
                    if not (last and tt == 3):
                        cpod[tt].append((lambda tt=tt: out_dma(tt), 0.0,
                                         f"od:{q}:{tt}"))
                if last:
                    # two-step skew: pv(tt) || norm+tp(tt-1) || c_proj(tt-2)
                    # so PE never waits a full DVE chain between subtiles
                    grp = [chunks[4 * tt:4 * tt + 4] for tt in range(4)]
                    newc = []
                    for step in range(6):
                        if step < 4:
                            newc.extend(grp[step][0:2])      # pv pair
                        if 1 <= step <= 4:
                            newc.extend(grp[step - 1][2:4])  # norm, tp
                        if step >= 2:
                            newc.extend(cpod[step - 2])
                    chunks[:] = newc
                else:
                    for tt in range(4):
                        late.extend(cpod[tt])
            return chunks, late

        # ---- main pipelined issue loop ----
        issued = set()

        # Queue A: Q/K projections of ALL tiles (critical path: enables
        # Act's late-tile exp work early).  Queue B: V projections and
        # retire work — drained in the Act-bound phase where PE has slack.
        workA = work
        workB = deque()

        def pop_work():
            src = workA if workA else workB
            chunk, cost, label = src.popleft()
            chunk()
            issued.add(label)
            return cost, label

        def pop_workB():
            chunk, cost, label = workB.popleft()
            chunk()
            issued.add(label)
            return cost, label

        # weight DMAs.  HWDGE is a single shared descriptor generator, so
        # issue order across queues IS landing order: K-hi first (the first
        # qk chain needs it), then x (tile 0, via qk_chunks' dma_x on sync),
        # then Q-hi and the lo parts on the Act queue (whose seq is busy
        # with LoadActFuncSet for the first ~2us anyway).  V and wp go on
        # the gpsimd SWDGE queue — pool is idle this early.
        # weight DMAs: K-hi first (first qk chain), x tile 0 next, then the
        # remaining tensors on the Act queue in need order.  All transfers
        # are [128, 2304]-contiguous on both sides (full DMA rate).
        nc.sync.dma_start(out=w8["k", 0], in_=wqkv_d["wkh"][:, :])
        workA.extend(qk_chunks(0))
        pop_work()   # x^T DMA of tile 0 — next on the sync queue
        nc.scalar.dma_start(out=w8["q", 0], in_=wqkv_d["wqh"][:, :])
        nc.scalar.dma_start(out=w8["k", 1], in_=wqkv_d["wkl"][:, :])
        nc.scalar.dma_start(out=w8["q", 1], in_=wqkv_d["wql"][:, :])
        nc.scalar.dma_start(out=w8["v", 0], in_=wqkv_d["wvh"][:, :])
        nc.scalar.dma_start(out=w8["v", 1], in_=wqkv_d["wvl"][:, :])
        nc.gpsimd.dma_start(out=wp, in_=wp_r)
        # warm the PE p-state while the first DMAs are in flight: dummy
        # matmuls on a const tile keep the array continuously busy so the
        # real Q/K projections start at full clock
        junk = const.tile([128, QBLK], F16)
        nc.vector.memset(junk, 0.0)
        for _ in range(WARMUP):
            jp = psS.tile([128, 2, QBLK], F32, tag="st", name="jp")
            MM(jp[:, 0, :], lhsT=ident, rhs=junk, start=True, stop=True)
        for _ in range(4):   # Q/K of pair 0 eagerly
            pop_work()
        for q in range(1, NQ):
            workA.extend(qk_chunks(q))

        # drain budgets per global S-batch index (measured-stall feedback);
        # records of what was actually drained are kept for the tuner.
        drained_rec = []
        marks = []   # PE event ordinal at the start of each S batch

        stages = [(q, p) for q in range(NQ) for p in range(NP)]
        if HOLDOUT and NQ > 1:
            # hold one small early stage for the end: its S/exp stream hides
            # the last big tile's c_proj, and its own retire tail is short
            stages.remove(HOLDOUT)
            stages.append(HOLDOUT)
        pend_late = []
        b = 0   # global S-batch index
        for i, (q, p) in enumerate(stages):
            if p == 0:
                workB.extend(v_chunks(q))
            # PE-order safety: this stage's Q/K groups must be issued first
            while (f"qk:{q}:{p}:0" not in issued
                   or f"qk:{q}:{p}:1" not in issued):
                pop_work()
            # pt-pool WAR safety: this stage's exp writes reuse the pt slot
            # of stage i-3 — its PV/norm chunks must already be issued, or
            # Act would wait on PE work scheduled after this stage
            if i >= 3:
                oq, op = stages[i - 3]
                while f"nm:{oq}:{op}:1" not in issued:
                    pop_workB()
            nk = 4 * (q + 1)
            pt = ptp.tile([128, nk, 2, QBLK], F16, tag="pt", name="pt")
            for k_i in range(nk):
                budget = gates[b] if gates is not None and b < len(gates) \
                    else DRAIN_NS
                spent = 0.0
                while workA or workB:
                    nxt = (workA or workB)[0][1]
                    if spent + max(nxt, 60.0) > budget + 200.0:
                        break
                    c, lab = pop_work()
                    spent += max(c, 60.0)
                drained_rec.append(spent)
                marks.append(pe_ord[0])
                s_batch(q, p, k_i, pt)
                b += 1
            if i == len(stages) - 1:
                # final stage: prior pair's transposes must precede its
                # per-subtile c_proj chains in issue order
                chunks, late = retire_chunks(q, p, pt, last=True)
                workB.extend(pend_late)
                workB.extend(chunks)
            else:
                chunks, late = retire_chunks(q, p, pt)
                workB.extend(chunks)
                workB.extend(pend_late)
            pend_late = late
        workB.extend(pend_late)
        while workA or workB:
            pop_work()

        build_nc.last_drained = drained_rec
        build_nc.last_marks = marks

    nc.compile()
    return nc


def make_in_map(x_b, w_attn, w_proj, g):
    """Per-core input arrays for batch slice x_b and head-group g."""
    import ml_dtypes
    E4 = ml_dtypes.float8_e4m3fn

    def shuf(w):
        # [768, 384] -> [r=128, (c2 j m)=2304]: row r holds the weights
        # for contraction rows c2*256 + j*128 + r, matching the on-device
        # DoubleRow tile layout with a fully contiguous DMA
        return np.ascontiguousarray(
            w.reshape(3, 2, 128, GC).transpose(2, 0, 1, 3).reshape(128, -1))

    sl = slice(g * GC, (g + 1) * GC)
    out = {}
    for nm, w in (("wq", w_attn[:, 0 * C:1 * C][:, sl]),
                  ("wk", w_attn[:, 1 * C:2 * C][:, sl]),
                  ("wv", w_attn[:, 2 * C:3 * C][:, sl])):
        ww = np.ascontiguousarray(w).astype(np.float32) * 32.0
        hi = ww.astype(E4)
        lo = (ww - hi.astype(np.float32)).astype(E4)
        out[f"w{nm[1]}h"] = shuf(hi)
        out[f"w{nm[1]}l"] = shuf(lo)
    xt = np.ascontiguousarray(x_b.T).astype(np.float32)
    xth = xt.astype(E4)
    out["xth"] = xth
    out["xtl"] = (xt - xth.astype(np.float32)).astype(E4)
    out["wp"] = np.ascontiguousarray(w_proj[sl, :]).astype(np.float16)
    return out


_NC_CACHE = {}


def _get_nc(T):
    if T not in _NC_CACHE:
        _NC_CACHE[T] = build_nc(T)
    return _NC_CACHE[T]


def kernel(x, w_attn, b_attn, w_proj, b_proj, _trace=False):
    from concourse.bass_utils import run_bass_kernel_spmd

    x = np.asarray(x, dtype=np.float32)
    w_attn = np.asarray(w_attn, dtype=np.float32)
    b_attn = np.asarray(b_attn, dtype=np.float32)
    w_proj = np.asarray(w_proj, dtype=np.float32)
    b_proj = np.asarray(b_proj, dtype=np.float32)
    B, T, _ = x.shape

    assert not np.any(b_attn[0:2 * C] != 0.0), \
        "nonzero q/k bias not supported by this kernel"

    nc = _get_nc(T)
    in_maps = []
    for b in range(B):
        for g in range(2):
            in_maps.append(make_in_map(x[b], w_attn, w_proj, g))
    res = run_bass_kernel_spmd(nc, in_maps, core_ids=list(range(2 * B)),
                               trace=_trace)
    outs = [np.asarray(r["out"], dtype=np.float32) for r in res.results]
    # softmax rows sum to 1, so the V-bias contribution is exactly
    # bv @ w_proj added to every token (not computed on device).
    bias_row = b_proj + b_attn[2 * C:3 * C] @ w_proj
    out = np.empty((B, T, C), dtype=np.float32)
    for b in range(B):
        out[b] = outs[2 * b] + outs[2 * b + 1] + bias_row[None, :]
    if _trace:
        kernel.last_result = res
    return out



# revision 76
# speedup vs baseline: 1.0031x; 1.0005x over previous
"""Causal self-attention (B=4 T=2048 C=768 H=12) on 8 trn2 cores — v3.

Sharding: core = (batch b, head-group g), g in {0,1} covering 6 heads
(3 pairs).  Host sums the two partial c_proj outputs per batch and adds
the exact bias row (b_proj + bv @ w_proj; softmax rows sum to 1).

v3 design (vs v2, 140850 -> 130486 ns):
  * hi/lo fp8 QKV projections: host splits x and 32*w_attn into e4m3
    hi + residual-lo (subnormal) pairs; three 256-contraction DoubleRow
    chains (hi.hi + hi.lo + lo.hi) in one psum accumulation run at 2x
    the fp16 rate with ~0.14% error.  1/32 unapplied in the evacuations.
  * S^T = K^T.Q per (pair, head) via fp8e4m3 DoubleRow matmuls at 0.5
    cycles/row (contraction-64 as 2 broadcast slots; exp scale 0.0625
    absorbs the doubling).
  * S evacuation split across engines (pool is barred from PSUM): act
    does true exp for ~58% of elements, dve does a 1-op fp16-bitcast
    schraudolph exp (tensor_scalar mult/add into an int16-bitcast view,
    1.8% RMS on its share; tile-scattered so end-to-end cost is ~5e-4).
    Per-q act shares lean higher for late tiles (dve carries retire).
  * causal masks on the pool engine (SBUF-only); y normalization via
    reciprocal + per-subtile tensor_scalar on dve; y^T via XBAR
    transpose-DMAs on the sync queue (PE/DVE freed); c_proj evacuation
    on dve early / act late (act idles in the tail).
  * chunk-queue software pipelining with a small (160ns) per-S-batch
    drain budget; weight/x DMA issue order tuned against the single
    serializing DMA_ENGINES device (K-hi, x0, Q-hi, los, V, wp).
"""

from collections import deque
from contextlib import ExitStack

import numpy as np

import concourse.bass as bass
import concourse.mybir as mybir
import concourse.tile as tile
from concourse import bacc
from concourse.masks import make_upper_triangular, make_identity

AF = mybir.ActivationFunctionType
ALU = mybir.AluOpType
F32 = mybir.dt.float32
F16 = mybir.dt.float16
F8 = mybir.dt.float8e4
I16 = mybir.dt.int16
DR = mybir.MatmulPerfMode.DoubleRow

# fp16-bitcast schraudolph exp approximation: exp(s*scale) ~=
# bitcast16(int16(A_SCHR*s + B_SCHR)).  C=-0.0575 minimizes RMS rel err
# (1.78%); +0.5 turns the trunc-on-int-store into round-to-nearest.
SCHR_SCALE = 0.0625          # fp8-DR doubled psum units
A_SCHR = float(np.float32(1024.0 * 1.4426950408889634 * SCHR_SCALE))
B_SCHR = float(np.float32(15 * 1024 + 1024.0 * (-0.0575) + 0.5))

# engine shares for the S-stream evacuation (fraction of elements).
# pool cannot touch PSUM (hard BIR rule), so the split is act (true exp,
# 0.833/elem) vs dve (schraudolph, 1.04/elem on fp32 psum).
SHARE = {"act": 0.60, "dve": 0.40}
# per-q act-share overrides: late (big) stages run while dve also carries
# the retire stream, so they lean harder on act
SHARE_BY_Q = {0: 0.55, 1: 0.58, 2: 0.63, 3: 0.61}
ALTERNATE = False   # strict act/dve alternation for S evacuation

C = 768          # model dim
D = 64           # head dim
HG = 6           # heads per core
NP = 3           # head pairs per core
GC = HG * D      # 384 group channels
CT = C // 128    # 6 contraction tiles
QBLK = 512       # query tile (psum bank)
KBLK = 128       # key tile

S_FP8 = True     # fp8e4m3 DoubleRow for the S matmul (else fp16)
WARMUP = 12      # PE p-state warmup matmuls at startup
HOLDOUT = None   # stage held to the end to shorten the tail (None = off)
DRAIN_NS = 160.0  # default per-S-batch PE-work drain budget (gates override)


def build_nc(T=2048, s_fp8=S_FP8, gates=None):
    NQ = T // QBLK
    NK = T // KBLK
    nc = bacc.Bacc(None)

    # x and w_attn arrive as hi/lo fp8 pairs: x ~= xth + xtl (lo holds the
    # quantization residual, subnormal-heavy), w_attn scaled by 32 so its
    # hi part uses the e4m3 normal range; the 1/32 is unapplied in the
    # q/k/v psum evacuations.  Projections run as three fp8 DoubleRow
    # chains (hi.hi + hi.lo + lo.hi) at 2x the fp16 matmul rate.
    xth_d = nc.dram_tensor("xth", [C, T], F8, kind="ExternalInput")
    xtl_d = nc.dram_tensor("xtl", [C, T], F8, kind="ExternalInput")
    # per-projection weights, host-PRE-SHUFFLED to [r=128, (c2 j m)] so
    # both DMA sides are 2304B-contiguous (full-rate; a 384B-row slice
    # transfer pays the <512B half-rate penalty)
    wqkv_d = {}
    for nm in ("wqh", "wql", "wkh", "wkl", "wvh", "wvl"):
        wqkv_d[nm] = nc.dram_tensor(nm, [128, 6 * GC], F8,
                                    kind="ExternalInput")
    wp_d = nc.dram_tensor("wp", [GC, C], F16, kind="ExternalInput")
    out_d = nc.dram_tensor("out", [T, C], F16, kind="ExternalOutput")
    W_UNSCALE = 1.0 / 32.0

    qk_dt = F8 if s_fp8 else F16
    exp_scale = 0.0625 if s_fp8 else 0.125

    with ExitStack() as ctx:
        tc = ctx.enter_context(tile.TileContext(nc))
        const = ctx.enter_context(tc.tile_pool(name="const", bufs=1))
        big = ctx.enter_context(tc.tile_pool(name="big", bufs=1))
        xtp = ctx.enter_context(tc.tile_pool(name="xtp", bufs=4))
        ptp = ctx.enter_context(tc.tile_pool(name="ptp", bufs=3))
        yqp = ctx.enter_context(tc.tile_pool(name="yqp", bufs=3))
        recp = ctx.enter_context(tc.tile_pool(name="recp", bufs=3))
        ytp = ctx.enter_context(tc.tile_pool(name="ytp", bufs=4))
        obp = ctx.enter_context(tc.tile_pool(name="obp", bufs=2))
        psS = ctx.enter_context(tc.tile_pool(name="psS", bufs=2, space="PSUM"))
        psY = ctx.enter_context(tc.tile_pool(name="psY", bufs=2, space="PSUM"))
        psQ = ctx.enter_context(tc.tile_pool(name="psQ", bufs=2, space="PSUM"))

        # constants.  ident first: the PE warmup matmuls depend on it, and
        # everything later on the pool queue (SWDGE descriptor generation,
        # mask builds) would delay it by ~3us.
        ident = const.tile([128, 128], F16)
        make_identity(nc, ident)
        mask2 = const.tile([128, 2, KBLK], F16)   # causal keep-mask, 2 heads
        make_upper_triangular(nc, mask2[:, 0, :], val=1.0, diag=True)
        make_upper_triangular(nc, mask2[:, 1, :], val=1.0, diag=True)

        # persistent.  weight/x layout for 256-contraction DoubleRow:
        # [r=128, ct2 in 3, j in 2, cols]; contraction c = ct2*256+j*128+r.
        CT2 = 3
        w8 = {}   # (proj, hl) -> [128, CT2, 2, GC] tile
        for proj in "qkv":
            for hl in (0, 1):
                w8[proj, hl] = big.tile([128, CT2, 2, GC], F8,
                                        name=f"w8{proj}{hl}")
        wp = big.tile([128, NP, C], F16)
        kt8 = big.tile([128, NP, T], qk_dt)
        qt8 = big.tile([128, NQ, NP, QBLK], qk_dt)
        # V with trailing ones column per (ktile, head): [k, 66] rows
        vs = big.tile([128, NK, HG, D + 2], F16)
        nc.gpsimd.memset(vs[:, :, :, D:D + 1], 1.0)

        xt_r = {0: xth_d[:, :].rearrange("(c2 j r) t -> r c2 j t",
                                         r=128, j=2),
                1: xtl_d[:, :].rearrange("(c2 j r) t -> r c2 j t",
                                         r=128, j=2)}
        # (xw)-chain list: (x hi/lo, w hi/lo)
        CHAINS = ((0, 0), (0, 1), (1, 0))
        wp_r = wp_d[:, :].rearrange("(p r) e -> r p e", r=128)
        out_r = out_d[:, :].rearrange("(q tt r) e -> q r tt e", tt=QBLK // KBLK,
                                      r=128)

        def dr_ap(ap):
            """[64, N] fp8 AP -> [64, 2, N] stride-0 DoubleRow operand."""
            return ap.unsqueeze(1).broadcast_to(
                [ap.shape[0], 2] + list(ap.shape[1:]))

        # ---- chunk helpers (each chunk = (callable, pe_ns), issued later) --
        PE_NS = 1.0 / 2.4   # ns per PE cycle at full speed
        work = deque()

        pe_ord = [0]   # PE event ordinal (Ldweights+Matmult pairs)

        def MM(*a, **k):
            pe_ord[0] += 2
            return nc.tensor.matmul(*a, **k)

        xtqs = {}

        def qk_chunks(q):
            """DMA + Q/K projection groups for tile q (critical early path)."""
            qs = q * QBLK

            def dma_x():
                xtq = xtp.tile([128, 2, CT2, 2, QBLK], F8, tag="xtq",
                               name="xtq")
                xtqs[q] = xtq
                for hl in (0, 1):   # hi first: half-0 chains need only hi
                    nc.sync.dma_start(out=xtq[:, hl, :, :, :],
                                      in_=xt_r[hl][:, :, :, qs:qs + QBLK])

            chunks = [(dma_x, 0.0, f"dx:{q}")]

            pss = {}

            def qk_part(p, which, half):
                if half == 0:
                    pss[(p, which)] = psQ.tile([128, QBLK], F32, tag="pq",
                                               name="pqk")
                ps = pss[(p, which)]
                proj = "q" if which == 0 else "k"
                # half 0: the hi.hi chain; half 1: the two cross chains
                chains = CHAINS[0:1] if half == 0 else CHAINS[1:3]
                for ci, (xs, ws) in enumerate(chains):
                    for ct2 in range(CT2):
                        first = half == 0 and ci == 0 and ct2 == 0
                        last = half == 1 and ci == len(chains) - 1 \
                            and ct2 == CT2 - 1
                        MM(ps, lhsT=w8[proj, ws][:, ct2, :,
                                                 p * 128:(p + 1) * 128],
                           rhs=xtqs[q][:, xs, ct2, :, :],
                           start=first, stop=last, perf_mode=DR)
                if half == 1:
                    del pss[(p, which)]
                    if which == 0:
                        nc.scalar.mul(qt8[:, q, p, :], ps, W_UNSCALE)
                    else:
                        nc.scalar.mul(kt8[:, p, qs:qs + QBLK], ps, W_UNSCALE)

            # half-0 (hi.hi) chains first within each pair: they need only
            # the hi tensors, which DMA-land first
            for p in range(NP):
                for half in (0, 1):
                    for which in (1, 0):   # K first
                        chunks.append(
                            (lambda p=p, w=which, h=half: qk_part(p, w, h),
                             (3 if half == 0 else 6) * 256 * PE_NS,
                             f"qk:{q}:{p}:{which}" if half else
                             f"qka:{q}:{p}:{which}"))
            return chunks

        def v_chunks(q):
            """V projection groups for tile q (needed by PV, not by S/exp)."""

            pss = {}

            def v_part(kl, half):
                k_i = 4 * q + kl
                if half == 0:
                    pss[kl] = psQ.tile([128, QBLK], F32, tag="pq", name="pv")
                ps = pss[kl]
                chains = CHAINS[0:1] if half == 0 else CHAINS[1:3]
                for ci, (xs, ws) in enumerate(chains):
                    for ct2 in range(CT2):
                        first = half == 0 and ci == 0 and ct2 == 0
                        last = half == 1 and ci == len(chains) - 1 \
                            and ct2 == CT2 - 1
                        MM(ps[:, 0:GC],
                           lhsT=xtqs[q][:, xs, ct2, :,
                                        kl * KBLK:(kl + 1) * KBLK],
                           rhs=w8["v", ws][:, ct2, :, :],
                           start=first, stop=last, perf_mode=DR)
                if half == 1:
                    del pss[kl]
                    nc.scalar.mul(
                        vs[:, k_i, :, 0:D],
                        ps[:, 0:GC].rearrange("r (h d) -> r h d", d=D),
                        W_UNSCALE)

            return [(lambda kl=kl, h=h: v_part(kl, h),
                     (3 if h == 0 else 6) * (GC // 2) * PE_NS,
                     f"v:{q}:{kl}" if h else f"va:{q}:{kl}")
                    for kl in range(4) for h in (0, 1)]

        # engine assignment for S-stream evacuation.  strict alternation:
        # consecutive evacuations on the same engine serialize (~1.1us
        # each) while alternating ones overlap, halving the S-stream
        # period.  ALT_PATTERN cycles act/dve; deficit mode weights by
        # SHARE instead.
        eng_credit = {"act": 0.0, "dve": 0.0}
        alt_state = [0]

        def pick_engine(elems, q=None):
            if ALTERNATE:
                e = ("act", "dve")[alt_state[0] % 2]
                alt_state[0] += 1
                return e
            a = SHARE_BY_Q.get(q, SHARE["act"])
            shares = {"act": a, "dve": 1.0 - a}
            for e in eng_credit:
                eng_credit[e] += shares[e] * elems
            best = max(eng_credit, key=lambda e: eng_credit[e])
            eng_credit[best] -= elems
            return best

        def s_evac(eng, dst, src):
            if eng == "act":
                nc.scalar.activation(dst, src, AF.Exp, scale=exp_scale)
            else:
                nc.vector.tensor_scalar(dst.bitcast(I16), src,
                                        A_SCHR, B_SCHR,
                                        op0=ALU.mult, op1=ALU.add)

        def s_batch(q, p, k_i, pt):
            """S^T (both heads) + exp + mask for one k-tile."""
            col0 = max(k_i - 4 * q, 0) * KBLK
            st = psS.tile([128, 2, QBLK], F32, tag="st", name="st")
            for s in range(2):
                hoff = D * s
                lhsT = kt8[hoff:hoff + D, p, k_i * KBLK:(k_i + 1) * KBLK]
                rhs = qt8[hoff:hoff + D, q, p, col0:QBLK]
                MM(st[:, s, col0:QBLK],
                   lhsT=dr_ap(lhsT), rhs=dr_ap(rhs),
                   start=True, stop=True, perf_mode=DR)
            if q == 3 and p == 2 and k_i >= 14:
                # program-final batches: act (the faster evacuator) so the
                # retire tail starts as early as possible
                eng = "act"
            else:
                eng = pick_engine(2 * (QBLK - col0), q)
            s_evac(eng, pt[:, k_i, :, col0:QBLK], st[:, :, col0:QBLK])
            if k_i >= 4 * q:   # diagonal tile: zero below-diagonal
                seg = pt[:, k_i, :, col0:col0 + KBLK]
                nc.gpsimd.tensor_mul(seg, seg, mask2)

        yts = {}
        tile_stages_done = {}

        def retire_chunks(q, p, pt, last=False):
            """PV + normalize chunks; transpose/c_proj go to `late` (they
            depend on DVE results of the PV chunks — spacing them a stage
            later avoids PE head-of-line stalls).  For the final stage
            (`last`), everything chains per token-subtile instead so the
            post-last-exp critical path covers one subtile, not four."""
            chunks = []
            late = []
            yas = {}
            yq4s = []

            def pv_group(s, tt):
                if tt == 0:
                    yas[s] = psY.tile([128, 4, KBLK], F32, tag="y", name="ya")
                ya = yas[s]
                h = p * 2 + s
                nkt = 4 * q + tt + 1
                for k_i in range(nkt):
                    MM(
                        ya[:, tt, 0:D + 1],
                        lhsT=pt[:, k_i, s, tt * KBLK:(tt + 1) * KBLK],
                        rhs=vs[:, k_i, h, 0:D + 1],
                        start=(k_i == 0), stop=(k_i == nkt - 1),
                        skip_group_check=True)

            def norm(s):
                # y * (1/rowsum), per token-subtile
                if not yq4s:
                    yq4s.append(yqp.tile([128, 4, 128], F16, tag="yq",
                                         name="yq"))
                ya = yas.pop(s)
                rec = recp.tile([128, 4], F32, tag="rec", name="rec")
                nc.vector.reciprocal_approx_fast(rec, ya[:, :, D:D + 1])
                for tt in range(4):
                    nc.vector.tensor_scalar(
                        yq4s[0][:, tt, s * D:(s + 1) * D], ya[:, tt, 0:D],
                        rec[:, tt:tt + 1], None, op0=ALU.mult)

            def transpose_all():
                # y^T via the DMA XBAR (14ns per 32x32 tile, runs on the
                # mostly-idle DMA engines; frees PE + DVE + a psum bank)
                if q not in yts:
                    yts[q] = ytp.tile([128, NP, QBLK], F16, tag="yt",
                                      name="yt")
                for tt in range(4):
                    nc.sync.dma_start_transpose(
                        yts[q][:, p, tt * KBLK:(tt + 1) * KBLK],
                        yq4s[0][:, tt, :])

            def norm_tt(tt):
                if not yq4s:
                    yq4s.append(yqp.tile([128, 4, 128], F16, tag="yq",
                                         name="yq"))
                rec = recp.tile([128, 2], F32, tag="rec", name="rec")
                for s in range(2):
                    nc.vector.reciprocal_approx_fast(
                        rec[:, s:s + 1], yas[s][:, tt, D:D + 1])
                    nc.vector.tensor_scalar(
                        yq4s[0][:, tt, s * D:(s + 1) * D],
                        yas[s][:, tt, 0:D],
                        rec[:, s:s + 1], None, op0=ALU.mult)

            def transpose_tt(tt):
                if q not in yts:
                    yts[q] = ytp.tile([128, NP, QBLK], F16, tag="yt",
                                      name="yt")
                tp = psQ.tile([128, KBLK], F16, tag="pq", name="tp")
                pe_ord[0] += 2
                nc.tensor.transpose(tp, yq4s[0][:, tt, :], ident)
                nc.vector.tensor_copy(
                    yts[q][:, p, tt * KBLK:(tt + 1) * KBLK], tp)

            if not last:
                for s in range(2):
                    for tt in range(4):
                        chunks.append((lambda s=s, tt=tt: pv_group(s, tt),
                                       (4 * q + tt + 1) * (D + 1) * PE_NS,
                                       f"pv:{q}:{p}:{s}:{tt}"))
                    chunks.append((lambda s=s: norm(s), 0.0,
                                   f"nm:{q}:{p}:{s}"))
                tps = [(transpose_all, 0.0, f"tp:{q}:{p}")]
            else:
                for tt in range(4):
                    for s in range(2):
                        chunks.append((lambda s=s, tt=tt: pv_group(s, tt),
                                       (4 * q + tt + 1) * (D + 1) * PE_NS,
                                       f"pv:{q}:{p}:{s}:{tt}"))
                    chunks.append((lambda tt=tt: norm_tt(tt), 0.0,
                                   f"nm:{q}:{p}:{tt // 3}"))
                    chunks.append((lambda tt=tt: transpose_tt(tt),
                                   128 * PE_NS + 70.0, f"tp:{q}:{p}:{tt}"))
                tps = []

            tile_stages_done[q] = tile_stages_done.get(q, 0) + 1
            late.extend(tps)
            if tile_stages_done[q] == NP:
                obs_local = {}

                def cproj2(tt, ec):
                    if q not in obs_local:
                        obs_local[q] = obp.tile([128, 4, C], F16, tag="ob",
                                                name="ob")
                    po = psQ.tile([128, QBLK], F32, tag="pq", name="po")
                    yt = yts[q]
                    for j in range(NP):
                        MM(
                            po[:, 0:GC],
                            lhsT=yt[:, j, tt * KBLK:(tt + 1) * KBLK],
                            rhs=wp[:, j, ec * GC:(ec + 1) * GC],
                            start=(j == 0), stop=(j == NP - 1))
                    dst = obs_local[q][:, tt, ec * GC:(ec + 1) * GC]
                    if q >= 2:
                        # late tiles: act is idle in the retire tail while
                        # dve is the bottleneck there
                        nc.scalar.copy(dst, po[:, 0:GC])
                    else:
                        nc.vector.tensor_copy(dst, po[:, 0:GC])

                def out_dma(tt):
                    nc.sync.dma_start(out=out_r[q][:, tt, :],
                                      in_=obs_local[q][:, tt, :])
                    if tt == 3:
                        yts.pop(q)
                        obs_local.pop(q)

                def out_dma_ec(ec):
                    # program-final subtile: per-ec DMA right after its
                    # evac shortens the very last evac->dma->sem chain
                    nc.sync.dma_start(
                        out=out_r[q][:, 3, ec * GC:(ec + 1) * GC],
                        in_=obs_local[q][:, 3, ec * GC:(ec + 1) * GC])
                    if ec == 1:
                        yts.pop(q)
                        obs_local.pop(q)

                cpod = [[] for _ in range(4)]
                for tt in range(4):
                    for ec in range(2):
                        cpod[tt].append(
                            (lambda tt=tt, ec=ec: cproj2(tt, ec),
                             NP * GC * PE_NS, f"cp:{q}:{tt}:{ec}"))
                        if last and tt == 3:
                            cpod[tt].append(
                                (lambda ec=ec: out_dma_ec(ec), 0.0,
                                 f"od:{q}:3:{ec}"))
                    if not (last and tt == 3):
                        cpod[tt].append((lambda tt=tt: out_dma(tt), 0.0,
                                         f"od:{q}:{tt}"))
                if last:
                    # two-step skew: pv(tt) || norm+tp(tt-1) || c_proj(tt-2)
                    # so PE never waits a full DVE chain between subtiles
                    grp = [chunks[4 * tt:4 * tt + 4] for tt in range(4)]
                    newc = []
                    for step in range(6):
                        if step < 4:
                            newc.extend(grp[step][0:2])      # pv pair
                        if 1 <= step <= 4:
                            newc.extend(grp[step - 1][2:4])  # norm, tp
                        if step >= 2:
                            newc.extend(cpod[step - 2])
                    chunks[:] = newc
                else:
                    for tt in range(4):
                        late.extend(cpod[tt])
            return chunks, late

        # ---- main pipelined issue loop ----
        issued = set()

        # Queue A: Q/K projections of ALL tiles (critical path: enables
        # Act's late-tile exp work early).  Queue B: V projections and
        # retire work — drained in the Act-bound phase where PE has slack.
        workA = work
        workB = deque()

        def pop_work():
            src = workA if workA else workB
            chunk, cost, label = src.popleft()
            chunk()
            issued.add(label)
            return cost, label

        def pop_workB():
            chunk, cost, label = workB.popleft()
            chunk()
            issued.add(label)
            return cost, label

        # weight DMAs.  HWDGE is a single shared descriptor generator, so
        # issue order across queues IS landing order: K-hi first (the first
        # qk chain needs it), then x (tile 0, via qk_chunks' dma_x on sync),
        # then Q-hi and the lo parts on the Act queue (whose seq is busy
        # with LoadActFuncSet for the first ~2us anyway).  V and wp go on
        # the gpsimd SWDGE queue — pool is idle this early.
        # weight DMAs: K-hi first (first qk chain), x tile 0 next, then the
        # remaining tensors on the Act queue in need order.  All transfers
        # are [128, 2304]-contiguous on both sides (full DMA rate).
        nc.sync.dma_start(out=w8["k", 0], in_=wqkv_d["wkh"][:, :])
        workA.extend(qk_chunks(0))
        pop_work()   # x^T DMA of tile 0 — next on the sync queue
        nc.scalar.dma_start(out=w8["q", 0], in_=wqkv_d["wqh"][:, :])
        nc.scalar.dma_start(out=w8["k", 1], in_=wqkv_d["wkl"][:, :])
        nc.scalar.dma_start(out=w8["q", 1], in_=wqkv_d["wql"][:, :])
        nc.scalar.dma_start(out=w8["v", 0], in_=wqkv_d["wvh"][:, :])
        nc.scalar.dma_start(out=w8["v", 1], in_=wqkv_d["wvl"][:, :])
        nc.gpsimd.dma_start(out=wp, in_=wp_r)
        # warm the PE p-state while the first DMAs are in flight: dummy
        # matmuls on a const tile keep the array continuously busy so the
        # real Q/K projections start at full clock
        junk = const.tile([128, QBLK], F16)
        nc.vector.memset(junk, 0.0)
        for _ in range(WARMUP):
            jp = psS.tile([128, 2, QBLK], F32, tag="st", name="jp")
            MM(jp[:, 0, :], lhsT=ident, rhs=junk, start=True, stop=True)
        for _ in range(4):   # Q/K of pair 0 eagerly
            pop_work()
        for q in range(1, NQ):
            workA.extend(qk_chunks(q))

        # drain budgets per global S-batch index (measured-stall feedback);
        # records of what was actually drained are kept for the tuner.
        drained_rec = []
        marks = []   # PE event ordinal at the start of each S batch

        stages = [(q, p) for q in range(NQ) for p in range(NP)]
        if HOLDOUT and NQ > 1:
            # hold one small early stage for the end: its S/exp stream hides
            # the last big tile's c_proj, and its own retire tail is short
            stages.remove(HOLDOUT)
            stages.append(HOLDOUT)
        pend_late = []
        b = 0   # global S-batch index
        for i, (q, p) in enumerate(stages):
            if p == 0:
                workB.extend(v_chunks(q))
            # PE-order safety: this stage's Q/K groups must be issued first
            while (f"qk:{q}:{p}:0" not in issued
                   or f"qk:{q}:{p}:1" not in issued):
                pop_work()
            # pt-pool WAR safety: this stage's exp writes reuse the pt slot
            # of stage i-3 — its PV/norm chunks must already be issued, or
            # Act would wait on PE work scheduled after this stage
            if i >= 3:
                oq, op = stages[i - 3]
                while f"nm:{oq}:{op}:1" not in issued:
                    pop_workB()
            nk = 4 * (q + 1)
            pt = ptp.tile([128, nk, 2, QBLK], F16, tag="pt", name="pt")
            for k_i in range(nk):
                budget = gates[b] if gates is not None and b < len(gates) \
                    else DRAIN_NS
                spent = 0.0
                while workA or workB:
                    nxt = (workA or workB)[0][1]
                    if spent + max(nxt, 60.0) > budget + 200.0:
                        break
                    c, lab = pop_work()
                    spent += max(c, 60.0)
                drained_rec.append(spent)
                marks.append(pe_ord[0])
                s_batch(q, p, k_i, pt)
                b += 1
            if i == len(stages) - 1:
                # final stage: prior pair's transposes must precede its
                # per-subtile c_proj chains in issue order
                chunks, late = retire_chunks(q, p, pt, last=True)
                workB.extend(pend_late)
                workB.extend(chunks)
            else:
                chunks, late = retire_chunks(q, p, pt)
                workB.extend(chunks)
                workB.extend(pend_late)
            pend_late = late
        workB.extend(pend_late)
        while workA or workB:
            pop_work()

        build_nc.last_drained = drained_rec
        build_nc.last_marks = marks

    nc.compile()
    return nc


def make_in_map(x_b, w_attn, w_proj, g):
    """Per-core input arrays for batch slice x_b and head-group g."""
    import ml_dtypes
    E4 = ml_dtypes.float8_e4m3fn

    def shuf(w):
        # [768, 384] -> [r=128, (c2 j m)=2304]: row r holds the weights
        # for contraction rows c2*256 + j*128 + r, matching the on-device
        # DoubleRow tile layout with a fully contiguous DMA
        return np.ascontiguousarray(
            w.reshape(3, 2, 128, GC).transpose(2, 0, 1, 3).reshape(128, -1))

    sl = slice(g * GC, (g + 1) * GC)
    out = {}
    for nm, w in (("wq", w_attn[:, 0 * C:1 * C][:, sl]),
                  ("wk", w_attn[:, 1 * C:2 * C][:, sl]),
                  ("wv", w_attn[:, 2 * C:3 * C][:, sl])):
        ww = np.ascontiguousarray(w).astype(np.float32) * 32.0
        hi = ww.astype(E4)
        lo = (ww - hi.astype(np.float32)).astype(E4)
        out[f"w{nm[1]}h"] = shuf(hi)
        out[f"w{nm[1]}l"] = shuf(lo)
    xt = np.ascontiguousarray(x_b.T).astype(np.float32)
    xth = xt.astype(E4)
    out["xth"] = xth
    out["xtl"] = (xt - xth.astype(np.float32)).astype(E4)
    out["wp"] = np.ascontiguousarray(w_proj[sl, :]).astype(np.float16)
    return out


_NC_CACHE = {}


def _get_nc(T):
    if T not in _NC_CACHE:
        _NC_CACHE[T] = build_nc(T)
    return _NC_CACHE[T]


def kernel(x, w_attn, b_attn, w_proj, b_proj, _trace=False):
    from concourse.bass_utils import run_bass_kernel_spmd

    x = np.asarray(x, dtype=np.float32)
    w_attn = np.asarray(w_attn, dtype=np.float32)
    b_attn = np.asarray(b_attn, dtype=np.float32)
    w_proj = np.asarray(w_proj, dtype=np.float32)
    b_proj = np.asarray(b_proj, dtype=np.float32)
    B, T, _ = x.shape

    assert not np.any(b_attn[0:2 * C] != 0.0), \
        "nonzero q/k bias not supported by this kernel"

    nc = _get_nc(T)
    in_maps = []
    for b in range(B):
        for g in range(2):
            in_maps.append(make_in_map(x[b], w_attn, w_proj, g))
    res = run_bass_kernel_spmd(nc, in_maps, core_ids=list(range(2 * B)),
                               trace=_trace)
    outs = [np.asarray(r["out"], dtype=np.float32) for r in res.results]
    # softmax rows sum to 1, so the V-bias contribution is exactly
    # bv @ w_proj added to every token (not computed on device).
    bias_row = b_proj + b_attn[2 * C:3 * C] @ w_proj
    out = np.empty((B, T, C), dtype=np.float32)
    for b in range(B):
        out[b] = outs[2 * b] + outs[2 * b + 1] + bias_row[None, :]
    if _trace:
        kernel.last_result = res
    return out



# revision 77
# speedup vs baseline: 1.0048x; 1.0018x over previous
"""Causal self-attention (B=4 T=2048 C=768 H=12) on 8 trn2 cores — v3.

Sharding: core = (batch b, head-group g), g in {0,1} covering 6 heads
(3 pairs).  Host sums the two partial c_proj outputs per batch and adds
the exact bias row (b_proj + bv @ w_proj; softmax rows sum to 1).

v3 design (vs v2, 140850 -> 130486 ns):
  * hi/lo fp8 QKV projections: host splits x and 32*w_attn into e4m3
    hi + residual-lo (subnormal) pairs; three 256-contraction DoubleRow
    chains (hi.hi + hi.lo + lo.hi) in one psum accumulation run at 2x
    the fp16 rate with ~0.14% error.  1/32 unapplied in the evacuations.
  * S^T = K^T.Q per (pair, head) via fp8e4m3 DoubleRow matmuls at 0.5
    cycles/row (contraction-64 as 2 broadcast slots; exp scale 0.0625
    absorbs the doubling).
  * S evacuation split across engines (pool is barred from PSUM): act
    does true exp for ~58% of elements, dve does a 1-op fp16-bitcast
    schraudolph exp (tensor_scalar mult/add into an int16-bitcast view,
    1.8% RMS on its share; tile-scattered so end-to-end cost is ~5e-4).
    Per-q act shares lean higher for late tiles (dve carries retire).
  * causal masks on the pool engine (SBUF-only); y normalization via
    reciprocal + per-subtile tensor_scalar on dve; y^T via XBAR
    transpose-DMAs on the sync queue (PE/DVE freed); c_proj evacuation
    on dve early / act late (act idles in the tail).
  * chunk-queue software pipelining with a small (160ns) per-S-batch
    drain budget; weight/x DMA issue order tuned against the single
    serializing DMA_ENGINES device (K-hi, x0, Q-hi, los, V, wp).
"""

from collections import deque
from contextlib import ExitStack

import numpy as np

import concourse.bass as bass
import concourse.mybir as mybir
import concourse.tile as tile
from concourse import bacc
from concourse.masks import make_upper_triangular, make_identity

AF = mybir.ActivationFunctionType
ALU = mybir.AluOpType
F32 = mybir.dt.float32
F16 = mybir.dt.float16
F8 = mybir.dt.float8e4
I16 = mybir.dt.int16
DR = mybir.MatmulPerfMode.DoubleRow

# fp16-bitcast schraudolph exp approximation: exp(s*scale) ~=
# bitcast16(int16(A_SCHR*s + B_SCHR)).  C=-0.0575 minimizes RMS rel err
# (1.78%); +0.5 turns the trunc-on-int-store into round-to-nearest.
SCHR_SCALE = 0.0625          # fp8-DR doubled psum units
A_SCHR = float(np.float32(1024.0 * 1.4426950408889634 * SCHR_SCALE))
B_SCHR = float(np.float32(15 * 1024 + 1024.0 * (-0.0575) + 0.5))

# engine shares for the S-stream evacuation (fraction of elements).
# pool cannot touch PSUM (hard BIR rule), so the split is act (true exp,
# 0.833/elem) vs dve (schraudolph, 1.04/elem on fp32 psum).
SHARE = {"act": 0.60, "dve": 0.40}
# per-q act-share overrides: late (big) stages run while dve also carries
# the retire stream, so they lean harder on act
SHARE_BY_Q = {0: 0.55, 1: 0.58, 2: 0.63, 3: 0.61}
ALTERNATE = False   # strict act/dve alternation for S evacuation

C = 768          # model dim
D = 64           # head dim
HG = 6           # heads per core
NP = 3           # head pairs per core
GC = HG * D      # 384 group channels
CT = C // 128    # 6 contraction tiles
QBLK = 512       # query tile (psum bank)
KBLK = 128       # key tile

S_FP8 = True     # fp8e4m3 DoubleRow for the S matmul (else fp16)
WARMUP = 12      # PE p-state warmup matmuls at startup
HOLDOUT = None   # stage held to the end to shorten the tail (None = off)
DRAIN_NS = 160.0  # default per-S-batch PE-work drain budget (gates override)


def build_nc(T=2048, s_fp8=S_FP8, gates=None):
    NQ = T // QBLK
    NK = T // KBLK
    nc = bacc.Bacc(None)

    # x and w_attn arrive as hi/lo fp8 pairs: x ~= xth + xtl (lo holds the
    # quantization residual, subnormal-heavy), w_attn scaled by 32 so its
    # hi part uses the e4m3 normal range; the 1/32 is unapplied in the
    # q/k/v psum evacuations.  Projections run as three fp8 DoubleRow
    # chains (hi.hi + hi.lo + lo.hi) at 2x the fp16 matmul rate.
    xth_d = nc.dram_tensor("xth", [C, T], F8, kind="ExternalInput")
    xtl_d = nc.dram_tensor("xtl", [C, T], F8, kind="ExternalInput")
    # per-projection weights, host-PRE-SHUFFLED to [r=128, (c2 j m)] so
    # both DMA sides are 2304B-contiguous (full-rate; a 384B-row slice
    # transfer pays the <512B half-rate penalty)
    wqkv_d = {}
    for nm in ("wqh", "wql", "wkh", "wkl", "wvh", "wvl"):
        wqkv_d[nm] = nc.dram_tensor(nm, [128, 6 * GC], F8,
                                    kind="ExternalInput")
    wp_d = nc.dram_tensor("wp", [GC, C], F16, kind="ExternalInput")
    out_d = nc.dram_tensor("out", [T, C], F16, kind="ExternalOutput")
    W_UNSCALE = 1.0 / 32.0

    qk_dt = F8 if s_fp8 else F16
    exp_scale = 0.0625 if s_fp8 else 0.125

    with ExitStack() as ctx:
        tc = ctx.enter_context(tile.TileContext(nc))
        const = ctx.enter_context(tc.tile_pool(name="const", bufs=1))
        big = ctx.enter_context(tc.tile_pool(name="big", bufs=1))
        xtp = ctx.enter_context(tc.tile_pool(name="xtp", bufs=4))
        ptp = ctx.enter_context(tc.tile_pool(name="ptp", bufs=3))
        yqp = ctx.enter_context(tc.tile_pool(name="yqp", bufs=3))
        recp = ctx.enter_context(tc.tile_pool(name="recp", bufs=3))
        ytp = ctx.enter_context(tc.tile_pool(name="ytp", bufs=4))
        obp = ctx.enter_context(tc.tile_pool(name="obp", bufs=2))
        psS = ctx.enter_context(tc.tile_pool(name="psS", bufs=2, space="PSUM"))
        psY = ctx.enter_context(tc.tile_pool(name="psY", bufs=2, space="PSUM"))
        psQ = ctx.enter_context(tc.tile_pool(name="psQ", bufs=2, space="PSUM"))

        # constants.  ident first: the PE warmup matmuls depend on it, and
        # everything later on the pool queue (SWDGE descriptor generation,
        # mask builds) would delay it by ~3us.
        ident = const.tile([128, 128], F16)
        make_identity(nc, ident)
        mask2 = const.tile([128, 2, KBLK], F16)   # causal keep-mask, 2 heads
        make_upper_triangular(nc, mask2[:, 0, :], val=1.0, diag=True)
        make_upper_triangular(nc, mask2[:, 1, :], val=1.0, diag=True)

        # persistent.  weight/x layout for 256-contraction DoubleRow:
        # [r=128, ct2 in 3, j in 2, cols]; contraction c = ct2*256+j*128+r.
        CT2 = 3
        w8 = {}   # (proj, hl) -> [128, CT2, 2, GC] tile
        for proj in "qkv":
            for hl in (0, 1):
                w8[proj, hl] = big.tile([128, CT2, 2, GC], F8,
                                        name=f"w8{proj}{hl}")
        wp = big.tile([128, NP, C], F16)
        kt8 = big.tile([128, NP, T], qk_dt)
        qt8 = big.tile([128, NQ, NP, QBLK], qk_dt)
        # V with trailing ones column per (ktile, head): [k, 66] rows
        vs = big.tile([128, NK, HG, D + 2], F16)
        nc.gpsimd.memset(vs[:, :, :, D:D + 1], 1.0)

        xt_r = {0: xth_d[:, :].rearrange("(c2 j r) t -> r c2 j t",
                                         r=128, j=2),
                1: xtl_d[:, :].rearrange("(c2 j r) t -> r c2 j t",
                                         r=128, j=2)}
        # (xw)-chain list: (x hi/lo, w hi/lo)
        CHAINS = ((0, 0), (0, 1), (1, 0))
        wp_r = wp_d[:, :].rearrange("(p r) e -> r p e", r=128)
        out_r = out_d[:, :].rearrange("(q tt r) e -> q r tt e", tt=QBLK // KBLK,
                                      r=128)

        def dr_ap(ap):
            """[64, N] fp8 AP -> [64, 2, N] stride-0 DoubleRow operand."""
            return ap.unsqueeze(1).broadcast_to(
                [ap.shape[0], 2] + list(ap.shape[1:]))

        # ---- chunk helpers (each chunk = (callable, pe_ns), issued later) --
        PE_NS = 1.0 / 2.4   # ns per PE cycle at full speed
        work = deque()

        pe_ord = [0]   # PE event ordinal (Ldweights+Matmult pairs)

        def MM(*a, **k):
            pe_ord[0] += 2
            return nc.tensor.matmul(*a, **k)

        xtqs = {}

        def qk_chunks(q):
            """DMA + Q/K projection groups for tile q (critical early path)."""
            qs = q * QBLK

            def dma_x():
                xtq = xtp.tile([128, 2, CT2, 2, QBLK], F8, tag="xtq",
                               name="xtq")
                xtqs[q] = xtq
                for hl in (0, 1):   # hi first: half-0 chains need only hi
                    nc.sync.dma_start(out=xtq[:, hl, :, :, :],
                                      in_=xt_r[hl][:, :, :, qs:qs + QBLK])

            chunks = [(dma_x, 0.0, f"dx:{q}")]

            pss = {}

            def qk_part(p, which, half):
                if half == 0:
                    pss[(p, which)] = psQ.tile([128, QBLK], F32, tag="pq",
                                               name="pqk")
                ps = pss[(p, which)]
                proj = "q" if which == 0 else "k"
                # half 0: the hi.hi chain; half 1: the two cross chains
                chains = CHAINS[0:1] if half == 0 else CHAINS[1:3]
                for ci, (xs, ws) in enumerate(chains):
                    for ct2 in range(CT2):
                        first = half == 0 and ci == 0 and ct2 == 0
                        last = half == 1 and ci == len(chains) - 1 \
                            and ct2 == CT2 - 1
                        MM(ps, lhsT=w8[proj, ws][:, ct2, :,
                                                 p * 128:(p + 1) * 128],
                           rhs=xtqs[q][:, xs, ct2, :, :],
                           start=first, stop=last, perf_mode=DR)
                if half == 1:
                    del pss[(p, which)]
                    if which == 0:
                        nc.scalar.mul(qt8[:, q, p, :], ps, W_UNSCALE)
                    else:
                        nc.scalar.mul(kt8[:, p, qs:qs + QBLK], ps, W_UNSCALE)

            # half-0 (hi.hi) chains first within each pair: they need only
            # the hi tensors, which DMA-land first
            for p in range(NP):
                for half in (0, 1):
                    for which in (1, 0):   # K first
                        chunks.append(
                            (lambda p=p, w=which, h=half: qk_part(p, w, h),
                             (3 if half == 0 else 6) * 256 * PE_NS,
                             f"qk:{q}:{p}:{which}" if half else
                             f"qka:{q}:{p}:{which}"))
            return chunks

        def v_chunks(q):
            """V projection groups for tile q (needed by PV, not by S/exp)."""

            pss = {}

            def v_part(kl, half):
                k_i = 4 * q + kl
                if half == 0:
                    pss[kl] = psQ.tile([128, QBLK], F32, tag="pq", name="pv")
                ps = pss[kl]
                chains = CHAINS[0:1] if half == 0 else CHAINS[1:3]
                for ci, (xs, ws) in enumerate(chains):
                    for ct2 in range(CT2):
                        first = half == 0 and ci == 0 and ct2 == 0
                        last = half == 1 and ci == len(chains) - 1 \
                            and ct2 == CT2 - 1
                        MM(ps[:, 0:GC],
                           lhsT=xtqs[q][:, xs, ct2, :,
                                        kl * KBLK:(kl + 1) * KBLK],
                           rhs=w8["v", ws][:, ct2, :, :],
                           start=first, stop=last, perf_mode=DR)
                if half == 1:
                    del pss[kl]
                    nc.scalar.mul(
                        vs[:, k_i, :, 0:D],
                        ps[:, 0:GC].rearrange("r (h d) -> r h d", d=D),
                        W_UNSCALE)

            return [(lambda kl=kl, h=h: v_part(kl, h),
                     (3 if h == 0 else 6) * (GC // 2) * PE_NS,
                     f"v:{q}:{kl}" if h else f"va:{q}:{kl}")
                    for kl in range(4) for h in (0, 1)]

        # engine assignment for S-stream evacuation.  strict alternation:
        # consecutive evacuations on the same engine serialize (~1.1us
        # each) while alternating ones overlap, halving the S-stream
        # period.  ALT_PATTERN cycles act/dve; deficit mode weights by
        # SHARE instead.
        # initial credit offset phase-shifts the whole assignment pattern;
        # +600 (elems toward act) is a measured dip of the chaotic schedule
        eng_credit = {"act": 600.0, "dve": -600.0}
        alt_state = [0]

        def pick_engine(elems, q=None):
            if ALTERNATE:
                e = ("act", "dve")[alt_state[0] % 2]
                alt_state[0] += 1
                return e
            a = SHARE_BY_Q.get(q, SHARE["act"])
            shares = {"act": a, "dve": 1.0 - a}
            for e in eng_credit:
                eng_credit[e] += shares[e] * elems
            best = max(eng_credit, key=lambda e: eng_credit[e])
            eng_credit[best] -= elems
            return best

        def s_evac(eng, dst, src):
            if eng == "act":
                nc.scalar.activation(dst, src, AF.Exp, scale=exp_scale)
            else:
                nc.vector.tensor_scalar(dst.bitcast(I16), src,
                                        A_SCHR, B_SCHR,
                                        op0=ALU.mult, op1=ALU.add)

        def s_batch(q, p, k_i, pt):
            """S^T (both heads) + exp + mask for one k-tile."""
            col0 = max(k_i - 4 * q, 0) * KBLK
            st = psS.tile([128, 2, QBLK], F32, tag="st", name="st")
            for s in range(2):
                hoff = D * s
                lhsT = kt8[hoff:hoff + D, p, k_i * KBLK:(k_i + 1) * KBLK]
                rhs = qt8[hoff:hoff + D, q, p, col0:QBLK]
                MM(st[:, s, col0:QBLK],
                   lhsT=dr_ap(lhsT), rhs=dr_ap(rhs),
                   start=True, stop=True, perf_mode=DR)
            if q == 3 and p == 2 and k_i >= 14:
                # program-final batches: act (the faster evacuator) so the
                # retire tail starts as early as possible
                eng = "act"
            else:
                eng = pick_engine(2 * (QBLK - col0), q)
            s_evac(eng, pt[:, k_i, :, col0:QBLK], st[:, :, col0:QBLK])
            if k_i >= 4 * q:   # diagonal tile: zero below-diagonal
                seg = pt[:, k_i, :, col0:col0 + KBLK]
                nc.gpsimd.tensor_mul(seg, seg, mask2)

        yts = {}
        tile_stages_done = {}

        def retire_chunks(q, p, pt, last=False):
            """PV + normalize chunks; transpose/c_proj go to `late` (they
            depend on DVE results of the PV chunks — spacing them a stage
            later avoids PE head-of-line stalls).  For the final stage
            (`last`), everything chains per token-subtile instead so the
            post-last-exp critical path covers one subtile, not four."""
            chunks = []
            late = []
            yas = {}
            yq4s = []

            def pv_group(s, tt):
                if tt == 0:
                    yas[s] = psY.tile([128, 4, KBLK], F32, tag="y", name="ya")
                ya = yas[s]
                h = p * 2 + s
                nkt = 4 * q + tt + 1
                for k_i in range(nkt):
                    MM(
                        ya[:, tt, 0:D + 1],
                        lhsT=pt[:, k_i, s, tt * KBLK:(tt + 1) * KBLK],
                        rhs=vs[:, k_i, h, 0:D + 1],
                        start=(k_i == 0), stop=(k_i == nkt - 1),
                        skip_group_check=True)

            def norm(s):
                # y * (1/rowsum), per token-subtile
                if not yq4s:
                    yq4s.append(yqp.tile([128, 4, 128], F16, tag="yq",
                                         name="yq"))
                ya = yas.pop(s)
                rec = recp.tile([128, 4], F32, tag="rec", name="rec")
                nc.vector.reciprocal_approx_fast(rec, ya[:, :, D:D + 1])
                for tt in range(4):
                    nc.vector.tensor_scalar(
                        yq4s[0][:, tt, s * D:(s + 1) * D], ya[:, tt, 0:D],
                        rec[:, tt:tt + 1], None, op0=ALU.mult)

            def transpose_all():
                # y^T via the DMA XBAR (14ns per 32x32 tile, runs on the
                # mostly-idle DMA engines; frees PE + DVE + a psum bank)
                if q not in yts:
                    yts[q] = ytp.tile([128, NP, QBLK], F16, tag="yt",
                                      name="yt")
                for tt in range(4):
                    nc.sync.dma_start_transpose(
                        yts[q][:, p, tt * KBLK:(tt + 1) * KBLK],
                        yq4s[0][:, tt, :])

            def norm_tt(tt):
                if not yq4s:
                    yq4s.append(yqp.tile([128, 4, 128], F16, tag="yq",
                                         name="yq"))
                rec = recp.tile([128, 2], F32, tag="rec", name="rec")
                for s in range(2):
                    nc.vector.reciprocal_approx_fast(
                        rec[:, s:s + 1], yas[s][:, tt, D:D + 1])
                    nc.vector.tensor_scalar(
                        yq4s[0][:, tt, s * D:(s + 1) * D],
                        yas[s][:, tt, 0:D],
                        rec[:, s:s + 1], None, op0=ALU.mult)

            def transpose_tt(tt):
                if q not in yts:
                    yts[q] = ytp.tile([128, NP, QBLK], F16, tag="yt",
                                      name="yt")
                tp = psQ.tile([128, KBLK], F16, tag="pq", name="tp")
                pe_ord[0] += 2
                nc.tensor.transpose(tp, yq4s[0][:, tt, :], ident)
                nc.vector.tensor_copy(
                    yts[q][:, p, tt * KBLK:(tt + 1) * KBLK], tp)

            if not last:
                for s in range(2):
                    for tt in range(4):
                        chunks.append((lambda s=s, tt=tt: pv_group(s, tt),
                                       (4 * q + tt + 1) * (D + 1) * PE_NS,
                                       f"pv:{q}:{p}:{s}:{tt}"))
                    chunks.append((lambda s=s: norm(s), 0.0,
                                   f"nm:{q}:{p}:{s}"))
                tps = [(transpose_all, 0.0, f"tp:{q}:{p}")]
            else:
                for tt in range(4):
                    for s in range(2):
                        chunks.append((lambda s=s, tt=tt: pv_group(s, tt),
                                       (4 * q + tt + 1) * (D + 1) * PE_NS,
                                       f"pv:{q}:{p}:{s}:{tt}"))
                    chunks.append((lambda tt=tt: norm_tt(tt), 0.0,
                                   f"nm:{q}:{p}:{tt // 3}"))
                    chunks.append((lambda tt=tt: transpose_tt(tt),
                                   128 * PE_NS + 70.0, f"tp:{q}:{p}:{tt}"))
                tps = []

            tile_stages_done[q] = tile_stages_done.get(q, 0) + 1
            late.extend(tps)
            if tile_stages_done[q] == NP:
                obs_local = {}

                def cproj2(tt, ec):
                    if q not in obs_local:
                        obs_local[q] = obp.tile([128, 4, C], F16, tag="ob",
                                                name="ob")
                    po = psQ.tile([128, QBLK], F32, tag="pq", name="po")
                    yt = yts[q]
                    for j in range(NP):
                        MM(
                            po[:, 0:GC],
                            lhsT=yt[:, j, tt * KBLK:(tt + 1) * KBLK],
                            rhs=wp[:, j, ec * GC:(ec + 1) * GC],
                            start=(j == 0), stop=(j == NP - 1))
                    dst = obs_local[q][:, tt, ec * GC:(ec + 1) * GC]
                    if q >= 2:
                        # late tiles: act is idle in the retire tail while
                        # dve is the bottleneck there
                        nc.scalar.copy(dst, po[:, 0:GC])
                    else:
                        nc.vector.tensor_copy(dst, po[:, 0:GC])

                def out_dma(tt):
                    nc.sync.dma_start(out=out_r[q][:, tt, :],
                                      in_=obs_local[q][:, tt, :])
                    if tt == 3:
                        yts.pop(q)
                        obs_local.pop(q)

                def out_dma_ec(ec):
                    # program-final subtile: per-ec DMA right after its
                    # evac shortens the very last evac->dma->sem chain
                    nc.sync.dma_start(
                        out=out_r[q][:, 3, ec * GC:(ec + 1) * GC],
                        in_=obs_local[q][:, 3, ec * GC:(ec + 1) * GC])
                    if ec == 1:
                        yts.pop(q)
                        obs_local.pop(q)

                cpod = [[] for _ in range(4)]
                for tt in range(4):
                    for ec in range(2):
                        cpod[tt].append(
                            (lambda tt=tt, ec=ec: cproj2(tt, ec),
                             NP * GC * PE_NS, f"cp:{q}:{tt}:{ec}"))
                        if last and tt == 3:
                            cpod[tt].append(
                                (lambda ec=ec: out_dma_ec(ec), 0.0,
                                 f"od:{q}:3:{ec}"))
                    if not (last and tt == 3):
                        cpod[tt].append((lambda tt=tt: out_dma(tt), 0.0,
                                         f"od:{q}:{tt}"))
                if last:
                    # two-step skew: pv(tt) || norm+tp(tt-1) || c_proj(tt-2)
                    # so PE never waits a full DVE chain between subtiles
                    grp = [chunks[4 * tt:4 * tt + 4] for tt in range(4)]
                    newc = []
                    for step in range(6):
                        if step < 4:
                            newc.extend(grp[step][0:2])      # pv pair
                        if 1 <= step <= 4:
                            newc.extend(grp[step - 1][2:4])  # norm, tp
                        if step >= 2:
                            newc.extend(cpod[step - 2])
                    chunks[:] = newc
                else:
                    for tt in range(4):
                        late.extend(cpod[tt])
            return chunks, late

        # ---- main pipelined issue loop ----
        issued = set()

        # Queue A: Q/K projections of ALL tiles (critical path: enables
        # Act's late-tile exp work early).  Queue B: V projections and
        # retire work — drained in the Act-bound phase where PE has slack.
        workA = work
        workB = deque()

        def pop_work():
            src = workA if workA else workB
            chunk, cost, label = src.popleft()
            chunk()
            issued.add(label)
            return cost, label

        def pop_workB():
            chunk, cost, label = workB.popleft()
            chunk()
            issued.add(label)
            return cost, label

        # weight DMAs.  HWDGE is a single shared descriptor generator, so
        # issue order across queues IS landing order: K-hi first (the first
        # qk chain needs it), then x (tile 0, via qk_chunks' dma_x on sync),
        # then Q-hi and the lo parts on the Act queue (whose seq is busy
        # with LoadActFuncSet for the first ~2us anyway).  V and wp go on
        # the gpsimd SWDGE queue — pool is idle this early.
        # weight DMAs: K-hi first (first qk chain), x tile 0 next, then the
        # remaining tensors on the Act queue in need order.  All transfers
        # are [128, 2304]-contiguous on both sides (full DMA rate).
        nc.sync.dma_start(out=w8["k", 0], in_=wqkv_d["wkh"][:, :])
        workA.extend(qk_chunks(0))
        pop_work()   # x^T DMA of tile 0 — next on the sync queue
        nc.scalar.dma_start(out=w8["q", 0], in_=wqkv_d["wqh"][:, :])
        nc.scalar.dma_start(out=w8["k", 1], in_=wqkv_d["wkl"][:, :])
        nc.scalar.dma_start(out=w8["q", 1], in_=wqkv_d["wql"][:, :])
        nc.scalar.dma_start(out=w8["v", 0], in_=wqkv_d["wvh"][:, :])
        nc.scalar.dma_start(out=w8["v", 1], in_=wqkv_d["wvl"][:, :])
        nc.gpsimd.dma_start(out=wp, in_=wp_r)
        # warm the PE p-state while the first DMAs are in flight: dummy
        # matmuls on a const tile keep the array continuously busy so the
        # real Q/K projections start at full clock
        junk = const.tile([128, QBLK], F16)
        nc.vector.memset(junk, 0.0)
        for _ in range(WARMUP):
            jp = psS.tile([128, 2, QBLK], F32, tag="st", name="jp")
            MM(jp[:, 0, :], lhsT=ident, rhs=junk, start=True, stop=True)
        for _ in range(4):   # Q/K of pair 0 eagerly
            pop_work()
        for q in range(1, NQ):
            workA.extend(qk_chunks(q))

        # drain budgets per global S-batch index (measured-stall feedback);
        # records of what was actually drained are kept for the tuner.
        drained_rec = []
        marks = []   # PE event ordinal at the start of each S batch

        stages = [(q, p) for q in range(NQ) for p in range(NP)]
        if HOLDOUT and NQ > 1:
            # hold one small early stage for the end: its S/exp stream hides
            # the last big tile's c_proj, and its own retire tail is short
            stages.remove(HOLDOUT)
            stages.append(HOLDOUT)
        pend_late = []
        b = 0   # global S-batch index
        for i, (q, p) in enumerate(stages):
            if p == 0:
                workB.extend(v_chunks(q))
            # PE-order safety: this stage's Q/K groups must be issued first
            while (f"qk:{q}:{p}:0" not in issued
                   or f"qk:{q}:{p}:1" not in issued):
                pop_work()
            # pt-pool WAR safety: this stage's exp writes reuse the pt slot
            # of stage i-3 — its PV/norm chunks must already be issued, or
            # Act would wait on PE work scheduled after this stage
            if i >= 3:
                oq, op = stages[i - 3]
                while f"nm:{oq}:{op}:1" not in issued:
                    pop_workB()
            nk = 4 * (q + 1)
            pt = ptp.tile([128, nk, 2, QBLK], F16, tag="pt", name="pt")
            for k_i in range(nk):
                budget = gates[b] if gates is not None and b < len(gates) \
                    else DRAIN_NS
                spent = 0.0
                while workA or workB:
                    nxt = (workA or workB)[0][1]
                    if spent + max(nxt, 60.0) > budget + 200.0:
                        break
                    c, lab = pop_work()
                    spent += max(c, 60.0)
                drained_rec.append(spent)
                marks.append(pe_ord[0])
                s_batch(q, p, k_i, pt)
                b += 1
            if i == len(stages) - 1:
                # final stage: prior pair's transposes must precede its
                # per-subtile c_proj chains in issue order
                chunks, late = retire_chunks(q, p, pt, last=True)
                workB.extend(pend_late)
                workB.extend(chunks)
            else:
                chunks, late = retire_chunks(q, p, pt)
                workB.extend(chunks)
                workB.extend(pend_late)
            pend_late = late
        workB.extend(pend_late)
        while workA or workB:
            pop_work()

        build_nc.last_drained = drained_rec
        build_nc.last_marks = marks

    nc.compile()
    return nc


def make_in_map(x_b, w_attn, w_proj, g):
    """Per-core input arrays for batch slice x_b and head-group g."""
    import ml_dtypes
    E4 = ml_dtypes.float8_e4m3fn

    def shuf(w):
        # [768, 384] -> [r=128, (c2 j m)=2304]: row r holds the weights
        # for contraction rows c2*256 + j*128 + r, matching the on-device
        # DoubleRow tile layout with a fully contiguous DMA
        return np.ascontiguousarray(
            w.reshape(3, 2, 128, GC).transpose(2, 0, 1, 3).reshape(128, -1))

    sl = slice(g * GC, (g + 1) * GC)
    out = {}
    for nm, w in (("wq", w_attn[:, 0 * C:1 * C][:, sl]),
                  ("wk", w_attn[:, 1 * C:2 * C][:, sl]),
                  ("wv", w_attn[:, 2 * C:3 * C][:, sl])):
        ww = np.ascontiguousarray(w).astype(np.float32) * 32.0
        hi = ww.astype(E4)
        lo = (ww - hi.astype(np.float32)).astype(E4)
        out[f"w{nm[1]}h"] = shuf(hi)
        out[f"w{nm[1]}l"] = shuf(lo)
    xt = np.ascontiguousarray(x_b.T).astype(np.float32)
    xth = xt.astype(E4)
    out["xth"] = xth
    out["xtl"] = (xt - xth.astype(np.float32)).astype(E4)
    out["wp"] = np.ascontiguousarray(w_proj[sl, :]).astype(np.float16)
    return out


_NC_CACHE = {}


def _get_nc(T):
    if T not in _NC_CACHE:
        _NC_CACHE[T] = build_nc(T)
    return _NC_CACHE[T]


def kernel(x, w_attn, b_attn, w_proj, b_proj, _trace=False):
    from concourse.bass_utils import run_bass_kernel_spmd

    x = np.asarray(x, dtype=np.float32)
    w_attn = np.asarray(w_attn, dtype=np.float32)
    b_attn = np.asarray(b_attn, dtype=np.float32)
    w_proj = np.asarray(w_proj, dtype=np.float32)
    b_proj = np.asarray(b_proj, dtype=np.float32)
    B, T, _ = x.shape

    assert not np.any(b_attn[0:2 * C] != 0.0), \
        "nonzero q/k bias not supported by this kernel"

    nc = _get_nc(T)
    in_maps = []
    for b in range(B):
        for g in range(2):
            in_maps.append(make_in_map(x[b], w_attn, w_proj, g))
    res = run_bass_kernel_spmd(nc, in_maps, core_ids=list(range(2 * B)),
                               trace=_trace)
    outs = [np.asarray(r["out"], dtype=np.float32) for r in res.results]
    # softmax rows sum to 1, so the V-bias contribution is exactly
    # bv @ w_proj added to every token (not computed on device).
    bias_row = b_proj + b_attn[2 * C:3 * C] @ w_proj
    out = np.empty((B, T, C), dtype=np.float32)
    for b in range(B):
        out[b] = outs[2 * b] + outs[2 * b + 1] + bias_row[None, :]
    if _trace:
        kernel.last_result = res
    return out

